# revision 2
# baseline (speedup 1.0000x reference)
"""Trainium2 Bass kernel for MCPRN (purpose-routed GRU-variant session recommender).

Pipeline (two SPMD launches on 8 NeuronCores):
  Launch 1 (scan): cores run (purpose p, batch-half h) PSRU scans, B_local=64.
     6 real slots + 2 duplicates. In-loop gi+gh bf16 matmuls accumulate in PSUM,
     fp32 elementwise state. Concentration weights (softmax over purposes of
     x . emb_purpose / tau, masked, eps-clamped) computed on device in fp32.
  Host gathers final hidden states hn[3, 128, 256] (bf16).
  Launch 2 (score): cores each score an item chunk (~6250 of 50001 items):
     scores[b, t] = sum_p tcw[t, p] * <hn[p, b, :], emb[t, :]>,
     tcw = softmax_p(emb @ emb_purpose.T). Softmax weights are broadcast
     across partitions with ones-matmuls; combine on DVE/GPSIMD.
"""

import numpy as np
import ml_dtypes

import concourse.bacc as bacc
import concourse.mybir as mybir
import concourse.tile as tile
from concourse.bass import ts, ds
from concourse.bass_utils import run_bass_kernel_spmd

F32 = mybir.dt.float32
BF16 = mybir.dt.bfloat16
AF = mybir.ActivationFunctionType
OP = mybir.AluOpType

N_ITEMS = 50001
DIM = 256
P = 3           # purposes
TAU = 0.1
S = 50
B = 128
EPS = 0.01
BH = 64         # batch half per scan core
SB = S * BH     # 3200 free elements of (step, batch) per scan core
NCORES = 8

# scoring chunking
T_PAD = 6272            # 49 * 128, per-core padded item count
N_CHUNK = 512
N_CHUNKS = T_PAD // N_CHUNK  # 12.25 -> handled as 12x512 + 1x128
CHUNK_SIZES = [512] * 12 + [128]
CHUNK_OFFS = np.cumsum([0] + CHUNK_SIZES).tolist()

CORE_PH = [(0, 0), (0, 1), (1, 0), (1, 1), (2, 0), (2, 1), (0, 0), (0, 1)]

_BF = ml_dtypes.bfloat16


# --------------------------------------------------------------------------
# Launch 1: scan
# --------------------------------------------------------------------------

def build_scan_nc():
    nc = bacc.Bacc("TRN2", target_bir_lowering=False, debug=False,
                   num_devices=NCORES)

    wiT_d = nc.dram_tensor("wiT", [128, 2, 768], BF16, kind="ExternalInput")
    whT_d = nc.dram_tensor("whT", [128, 2, 768], BF16, kind="ExternalInput")
    xT_d = nc.dram_tensor("xT", [128, 2, SB], BF16, kind="ExternalInput")
    xTf_d = nc.dram_tensor("xTf", [128, 2, SB], F32, kind="ExternalInput")
    pT_d = nc.dram_tensor("pT", [128, 2, 3], F32, kind="ExternalInput")
    mask_d = nc.dram_tensor("mask", [128, SB // 128], F32, kind="ExternalInput")
    b_ri_d = nc.dram_tensor("b_ri", [128, 4, BH], F32, kind="ExternalInput")
    b_hn_d = nc.dram_tensor("b_hn", [128, 2, BH], F32, kind="ExternalInput")
    b_in_d = nc.dram_tensor("b_in", [128, 2, BH], F32, kind="ExternalInput")
    psel_d = nc.dram_tensor("psel", [1], mybir.dt.int32, kind="ExternalInput")
    hn_out = nc.dram_tensor("hn_out", [128, 2 * BH], BF16, kind="ExternalOutput")
    # intermediate DRAM for the cf broadcast roundtrip
    cf_lin = nc.dram_tensor("cf_lin", [SB // 128, 128], F32)

    NT = SB // 128  # 25 (s,b)-tiles for concen

    with tile.TileContext(nc) as tc:
        with (
            tc.tile_pool(name="consts", bufs=1) as consts,
            tc.tile_pool(name="cw", bufs=1) as cw,
            tc.tile_pool(name="cpsum", bufs=1, space="PSUM") as cpsum,
            tc.tile_pool(name="spsum", bufs=2, space="PSUM") as spsum,
            tc.tile_pool(name="ew", bufs=3) as ew,
            tc.tile_pool(name="hpool", bufs=2) as hpool,
        ):
            wiT = consts.tile_from(wiT_d.ap())
            whT = consts.tile_from(whT_d.ap())
            xT = consts.tile_from(xT_d.ap())
            xTf = consts.tile_from(xTf_d.ap())
            pT = consts.tile_from(pT_d.ap())
            mask = consts.tile_from(mask_d.ap())
            b_ri = consts.tile_from(b_ri_d.ap())
            b_hn = consts.tile_from(b_hn_d.ap())
            b_in = consts.tile_from(b_in_d.ap())

            # ---------------- concen -> cf_rep ----------------
            # logits in [sb-part, p-free] layout, 25 tiles packed in one bank
            ps_s = cpsum.tile([128, NT, 3], F32)
            for tt in range(NT):
                for k in range(2):
                    nc.tensor.matmul(
                        ps_s[:, tt, :],
                        xTf[:, k, ts(tt, 128)],
                        pT[:, k, :],
                        start=(k == 0), stop=(k == 1),
                    )
            # softmax over the 3-wide innermost dim (logits/TAU via scale)
            e_s = cw.tile([128, NT, 3], F32)
            nc.scalar.activation(e_s[:], ps_s[:], AF.Exp, scale=1.0 / TAU)
            den = cw.tile([128, NT], F32)
            nc.vector.tensor_reduce(den[:], e_s[:], mybir.AxisListType.X, OP.add)
            rden = cw.tile([128, NT], F32)
            nc.vector.reciprocal(rden[:], den[:])
            cnorm = cw.tile([128, NT, 3], F32)
            nc.vector.tensor_tensor(
                cnorm[:], e_s[:],
                rden[:, :, None].to_broadcast((128, NT, 3)), OP.mult)
            # apply padding mask, then eps clamp: cf = c * (c >= EPS)
            cm = cw.tile([128, NT, 3], F32)
            nc.vector.tensor_tensor(
                cm[:], cnorm[:],
                mask[:, :, None].to_broadcast((128, NT, 3)), OP.mult)
            ge = cw.tile([128, NT, 3], F32)
            nc.vector.tensor_scalar(ge[:], cm[:], EPS, None, OP.is_ge)
            cf3 = cw.tile([128, NT, 3], F32)
            nc.vector.tensor_tensor(cf3[:], cm[:], ge[:], OP.mult)

            # select this core's purpose column via indirect copy:
            # psel is 0/1/2; use dynamic slice on the innermost dim.
            # Simplest portable approach: compute all three, DMA out with a
            # dynamic offset. bass dynamic slice: use ds() with register is
            # complex; instead host bakes purpose by pre-permuting pT columns
            # so that column 0 is always this core's purpose.
            cf_p = cf3[:, :, 0]  # [128, NT] (host permuted purposes)

            # DRAM roundtrip to broadcast over partitions:
            # cf_lin[tt, pi] = cf_p[pi, tt]
            nc.sync.dma_start(cf_lin.ap().rearrange("t p -> p t"), cf_p)
            cf_rep = cw.tile([128, SB], F32)
            nc.sync.dma_start(
                cf_rep[:],
                cf_lin.ap().rearrange("t p -> (t p)")[None, :].to_broadcast((128, SB)),
            )

            # ---------------- the scan ----------------
            h = hpool.tile([128, 2, BH], F32, tag="h")
            nc.vector.memset(h[:], 0.0)
            h_bf = hpool.tile([128, 2, BH], BF16, tag="hbf")
            nc.vector.memset(h_bf[:], 0.0)

            for t in range(S):
                xsl = xT[:, :, ts(t, BH)]  # [128, 2, BH]
                ps_ri = spsum.tile([128, 4, BH], F32, tag="ri")
                ps_ghn = spsum.tile([128, 2, BH], F32, tag="ghn")
                ps_gin = spsum.tile([128, 2, BH], F32, tag="gin")

                # n-gate input part first (no h dependency)
                for j in range(2):
                    for k in range(2):
                        nc.tensor.matmul(
                            ps_gin[:, j, :], wiT[:, k, ds(512 + j * 128, 128)],
                            xsl[:, k, :], start=(k == 0), stop=(k == 1))
                # r,i gates: ih + hh accumulate
                for j in range(4):
                    for k in range(2):
                        nc.tensor.matmul(
                            ps_ri[:, j, :], wiT[:, k, ts(j, 128)],
                            xsl[:, k, :], start=(k == 0), stop=False)
                    for k in range(2):
                        nc.tensor.matmul(
                            ps_ri[:, j, :], whT[:, k, ts(j, 128)],
                            h_bf[:, k, :], start=False, stop=(k == 1))
                # n-gate hidden part
                for j in range(2):
                    for k in range(2):
                        nc.tensor.matmul(
                            ps_ghn[:, j, :], whT[:, k, ds(512 + j * 128, 128)],
                            h_bf[:, k, :], start=(k == 0), stop=(k == 1))

                # elementwise
                ginb = ew.tile([128, 2, BH], F32, tag="ginb")
                nc.vector.tensor_tensor(ginb[:], ps_gin[:], b_in[:], OP.add)
                ghnb = ew.tile([128, 2, BH], F32, tag="ghnb")
                nc.vector.tensor_tensor(ghnb[:], ps_ghn[:], b_hn[:], OP.add)
                rib = ew.tile([128, 4, BH], F32, tag="rib")
                nc.vector.tensor_tensor(rib[:], ps_ri[:], b_ri[:], OP.add)
                ri = ew.tile([128, 4, BH], BF16, tag="ri")
                nc.scalar.activation(ri[:], rib[:], AF.Sigmoid)
                u1 = ew.tile([128, 2, BH], F32, tag="u1")
                nc.vector.tensor_tensor(u1[:], ri[:, 0:2, :], ghnb[:], OP.mult)
                u2 = ew.tile([128, 2, BH], F32, tag="u2")
                nc.vector.tensor_tensor(u2[:], u1[:], ginb[:], OP.add)
                n_t = ew.tile([128, 2, BH], F32, tag="n")
                nc.scalar.activation(n_t[:], u2[:], AF.Tanh)
                a_t = ew.tile([128, 2, BH], F32, tag="a")
                nc.gpsimd.tensor_tensor(
                    a_t[:], ri[:, 2:4, :],
                    cf_rep[:, None, ts(t, BH)].to_broadcast((128, 2, BH)),
                    OP.mult)
                dn = ew.tile([128, 2, BH], F32, tag="dn")
                nc.vector.tensor_tensor(dn[:], n_t[:], h[:], OP.subtract)
                prod = ew.tile([128, 2, BH], F32, tag="prod")
                nc.vector.tensor_tensor(prod[:], a_t[:], dn[:], OP.mult)
                h_new = hpool.tile([128, 2, BH], F32, tag="h")
                nc.vector.tensor_tensor(h_new[:], h[:], prod[:], OP.add)
                h_bf_new = hpool.tile([128, 2, BH], BF16, tag="hbf")
                nc.vector.tensor_copy(h_bf_new[:], h_new[:])
                h = h_new
                h_bf = h_bf_new

            nc.sync.dma_start(hn_out.ap().rearrange("p (k b) -> p k b", k=2), h_bf[:])

    nc.compile()
    return nc


def scan_host_inputs(seq, emb, emb_purpose, w_ih, w_hh, b_ih, b_hh):
    """Build per-core input maps for the scan launch."""
    seq = np.asarray(seq)
    xg = emb[seq]                      # [S, B, D] gather (input staging)
    in_maps = []
    for c in range(NCORES):
        p, h = CORE_PH[c]
        sl = slice(h * BH, (h + 1) * BH)
        xh = xg[:, sl, :]              # [S, BH, D]
        xT = np.ascontiguousarray(
            xh.transpose(2, 0, 1).reshape(2, 128, SB).transpose(1, 0, 2))
        wiT = np.ascontiguousarray(
            w_ih[p].T.reshape(2, 128, 768).transpose(1, 0, 2))
        whT = np.ascontiguousarray(
            w_hh[p].T.reshape(2, 128, 768).transpose(1, 0, 2))
        # permute purposes so column 0 is this core's purpose
        perm = [p, (p + 1) % 3, (p + 2) % 3]
        pT = np.ascontiguousarray(
            emb_purpose[perm].T.reshape(2, 128, 3).transpose(1, 0, 2))
        m = (seq[:, sl] != 0).astype(np.float32).reshape(SB)
        mask = np.ascontiguousarray(m.reshape(SB // 128, 128).T)
        bsum = (b_ih[p] + b_hh[p])[:512]
        b_ri = np.broadcast_to(
            bsum.reshape(4, 128).T[:, :, None], (128, 4, BH)).copy()
        b_hn = np.broadcast_to(
            b_hh[p][512:].reshape(2, 128).T[:, :, None], (128, 2, BH)).copy()
        b_in = np.broadcast_to(
            b_ih[p][512:].reshape(2, 128).T[:, :, None], (128, 2, BH)).copy()
        in_maps.append({
            "wiT": wiT.astype(_BF), "whT": whT.astype(_BF),
            "xT": xT.astype(_BF), "xTf": xT.astype(np.float32),
            "pT": pT.astype(np.float32), "mask": mask,
            "b_ri": b_ri.astype(np.float32), "b_hn": b_hn.astype(np.float32),
            "b_in": b_in.astype(np.float32),
            "psel": np.array([p], np.int32),
        })
    return in_maps


# --------------------------------------------------------------------------
# Launch 2: scoring
# --------------------------------------------------------------------------

def build_score_nc():
    nc = bacc.Bacc("TRN2", target_bir_lowering=False, debug=False,
                   num_devices=NCORES)

    hT6_d = nc.dram_tensor("hT6", [128, 6, 128], BF16, kind="ExternalInput")
    eT_d = nc.dram_tensor("eT", [128, 2, T_PAD], BF16, kind="ExternalInput")
    pT_d = nc.dram_tensor("pTs", [128, 2, 3], BF16, kind="ExternalInput")
    sel_d = nc.dram_tensor("sel", [128, 4, 128], BF16, kind="ExternalInput")
    scores_d = nc.dram_tensor("scores", [128, T_PAD], F32, kind="ExternalOutput")

    with tile.TileContext(nc) as tc:
        with (
            tc.tile_pool(name="consts", bufs=1) as consts,
            tc.tile_pool(name="spsum", bufs=1, space="PSUM") as spsum,
            tc.tile_pool(name="epsum", bufs=1, space="PSUM") as epsum,
            tc.tile_pool(name="work", bufs=3) as work,
            tc.tile_pool(name="outp", bufs=3) as outp,
        ):
            hT6 = consts.tile_from(hT6_d.ap())
            eT = consts.tile_from(eT_d.ap())
            pT = consts.tile_from(pT_d.ap())
            sel = consts.tile_from(sel_d.ap())

            # exp(logits) for all purposes, padded to 128 partitions (rows 3+
            # stay zero -> exp never evaluated there since we only write 0:3)
            sE = consts.tile([128, T_PAD], BF16)
            nc.vector.memset(sE[:], 0.0)

            for ci, (c0, cs) in enumerate(zip(CHUNK_OFFS[:-1], CHUNK_SIZES)):
                ps_s = spsum.tile([128, N_CHUNK], F32, tag="logit")
                for k in range(2):
                    nc.tensor.matmul(
                        ps_s[0:3, :cs], pT[:, k, :], eT[:, k, ds(c0, cs)],
                        start=(k == 0), stop=(k == 1))
                nc.scalar.activation(sE[0:3, ds(c0, cs)], ps_s[0:3, :cs], AF.Exp)

            for ci, (c0, cs) in enumerate(zip(CHUNK_OFFS[:-1], CHUNK_SIZES)):
                # per-purpose partial scores
                ps_P = [epsum.tile([128, N_CHUNK], F32, tag=f"P{p}", name=f"psP{p}")
                        for p in range(3)]
                for p in range(3):
                    for k in range(2):
                        nc.tensor.matmul(
                            ps_P[p][:, :cs], hT6[:, p * 2 + k, :],
                            eT[:, k, ds(c0, cs)], start=(k == 0), stop=(k == 1))
                # broadcast exp rows and their sum across partitions
                ps_E = [epsum.tile([128, N_CHUNK], F32, tag=f"E{j}", name=f"psE{j}")
                        for j in range(4)]
                for j in range(4):
                    nc.tensor.matmul(
                        ps_E[j][:, :cs], sel[:, j, :], sE[:, ds(c0, cs)],
                        start=True, stop=True)
                # reciprocal of the softmax denominator (broadcast layout)
                den_bf = work.tile([128, N_CHUNK], BF16, tag="denbf")
                nc.vector.tensor_copy(den_bf[:, :cs], ps_E[3][:, :cs])
                rden = work.tile([128, N_CHUNK], F32, tag="rden")
                nc.vector.reciprocal(rden[:, :cs], den_bf[:, :cs])
                # evacuate E_p via scalar engine, combine on vector/gpsimd
                e_sb = [work.tile([128, N_CHUNK], BF16, tag=f"esb{p}", name=f"esb{p}")
                        for p in range(3)]
                for p in range(3):
                    nc.scalar.copy(e_sb[p][:, :cs], ps_E[p][:, :cs])
                t_p = [work.tile([128, N_CHUNK], BF16, tag=f"tp{p}", name=f"tp{p}")
                       for p in range(3)]
                for p in range(3):
                    nc.vector.tensor_tensor(
                        t_p[p][:, :cs], ps_P[p][:, :cs], e_sb[p][:, :cs], OP.mult)
                s01 = work.tile([128, N_CHUNK], BF16, tag="s01")
                nc.gpsimd.tensor_tensor(
                    s01[:, :cs], t_p[0][:, :cs], t_p[1][:, :cs], OP.add)
                s012 = work.tile([128, N_CHUNK], BF16, tag="s012")
                nc.gpsimd.tensor_tensor(
                    s012[:, :cs], s01[:, :cs], t_p[2][:, :cs], OP.add)
                out_c = outp.tile([128, N_CHUNK], F32, tag="out")
                nc.vector.tensor_tensor(
                    out_c[:, :cs], s012[:, :cs], rden[:, :cs], OP.mult)
                nc.sync.dma_start(scores_d.ap()[:, ds(c0, cs)], out_c[:, :cs])

    nc.compile()
    return nc


def score_host_inputs(hn_bf, emb, emb_purpose):
    """hn_bf: [3, 2, 128, 64*2] assembled per (p, k) -> [128d, 6, 128b]."""
    embT = emb.T.astype(_BF)  # [256, 50001]
    pT = np.ascontiguousarray(
        emb_purpose.T.reshape(2, 128, 3).transpose(1, 0, 2)).astype(_BF)
    sel = np.zeros((128, 4, 128), np.float32)
    for p in range(3):
        sel[p, p, :] = 1.0
        sel[p, 3, :] = 1.0
    sel = sel.astype(_BF)

    # chunk boundaries over 50001 items
    base = N_ITEMS // NCORES            # 6250
    rem = N_ITEMS - base * NCORES       # 1
    bounds = []
    s0 = 0
    for c in range(NCORES):
        n = base + (1 if c < rem else 0)
        bounds.append((s0, s0 + n))
        s0 += n

    in_maps = []
    for c in range(NCORES):
        lo, hi = bounds[c]
        n = hi - lo
        eT = np.zeros((128, 2, T_PAD), _BF)
        chunk = embT[:, lo:hi]  # [256, n]
        eT[:, :, :n] = chunk.reshape(2, 128, n).transpose(1, 0, 2)
        in_maps.append({
            "hT6": hn_bf, "eT": eT, "pTs": pT, "sel": sel,
        })
    return in_maps, bounds


# --------------------------------------------------------------------------
# Entry point
# --------------------------------------------------------------------------

_SCAN_NC = None
_SCORE_NC = None


def _get_ncs():
    global _SCAN_NC, _SCORE_NC
    if _SCAN_NC is None:
        _SCAN_NC = build_scan_nc()
    if _SCORE_NC is None:
        _SCORE_NC = build_score_nc()
    return _SCAN_NC, _SCORE_NC


def kernel(seq, emb, emb_purpose, w_ih, w_hh, b_ih, b_hh):
    seq = np.asarray(seq)
    emb = np.asarray(emb, np.float32)
    emb_purpose = np.asarray(emb_purpose, np.float32)
    w_ih = np.asarray(w_ih, np.float32)
    w_hh = np.asarray(w_hh, np.float32)
    b_ih = np.asarray(b_ih, np.float32)
    b_hh = np.asarray(b_hh, np.float32)

    scan_nc, score_nc = _get_ncs()

    scan_ins = scan_host_inputs(seq, emb, emb_purpose, w_ih, w_hh, b_ih, b_hh)
    res1 = run_bass_kernel_spmd(scan_nc, scan_ins, core_ids=list(range(NCORES)))

    # assemble hT6[128d, 6(p,k), 128b]
    hT6 = np.zeros((128, 6, 128), _BF)
    for c in range(6):
        p, h = CORE_PH[c]
        sl = res1.results[c]["hn_out"].reshape(128, 2, BH)
        for k in range(2):
            hT6[:, p * 2 + k, h * BH:(h + 1) * BH] = sl[:, k, :]

    score_ins, bounds = score_host_inputs(hT6, emb, emb_purpose)
    res2 = run_bass_kernel_spmd(score_nc, score_ins, core_ids=list(range(NCORES)))

    scores = np.empty((B, N_ITEMS), np.float32)
    for c in range(NCORES):
        lo, hi = bounds[c]
        scores[:, lo:hi] = res2.results[c]["scores"][:, : hi - lo]
    return scores


# revision 11
# speedup vs baseline: 1.1011x; 1.1011x over previous
"""Trainium2 Bass kernel for MCPRN (purpose-routed GRU-variant session recommender).

Pipeline (two SPMD launches on 8 NeuronCores):
  Launch 1 (scan): cores run (purpose p, batch-half h) PSRU scans, B_local=64.
     6 real slots + 2 duplicates. bf16 matmuls, fp32 elementwise state.
     Biases enter PSUM via K=1 ones-matmuls; the x-side (input) matmuls are
     batched 4 steps at a time (N=256) and the recurrent matmuls accumulate
     into the same PSUM group. Concentration weights (softmax over purposes
     of x . emb_purpose / tau, masked, eps-clamped) are computed on device in
     fp32 and broadcast across partitions via a DRAM roundtrip.
  Host gathers final hidden states hn[3, 128, 256] (bf16).
  Launch 2 (score): cores each score an item chunk (~6250 of 50001 items):
     scores[b, t] = sum_p tcw[t, p] * <hn[p, b, :], emb[t, :]>,
     tcw = softmax_p(emb @ emb_purpose.T). Softmax weights are broadcast
     across partitions with ones-matmuls; combine on DVE/GPSIMD.
"""

import numpy as np
import ml_dtypes

import concourse.bacc as bacc
import concourse.mybir as mybir
import concourse.tile as tile
from concourse.bass import ts, ds
from concourse.bass_utils import run_bass_kernel_spmd

F32 = mybir.dt.float32
BF16 = mybir.dt.bfloat16
AF = mybir.ActivationFunctionType
OP = mybir.AluOpType

N_ITEMS = 50001
DIM = 256
TAU = 0.1
S = 50
B = 128
EPS = 0.01
BH = 64         # batch half per scan core
SB = S * BH     # 3200 (step, batch) elements per scan core
NCORES = 8
GS = 8          # steps per x-side matmul group (8*64 = 512 f32 = 1 PSUM bank)
GROUPS = [(g, min(GS, S - g)) for g in range(0, S, GS)]

# scoring chunking
T_PAD = 6272            # 49 * 128, per-core padded item count
N_CHUNK = 512
CHUNK_SIZES = [512] * 12 + [128]
CHUNK_OFFS = np.cumsum([0] + CHUNK_SIZES).tolist()

CORE_PH = [(0, 0), (0, 1), (1, 0), (1, 1), (2, 0), (2, 1), (0, 0), (0, 1)]

_BF = ml_dtypes.bfloat16


# --------------------------------------------------------------------------
# Launch 1: scan
# --------------------------------------------------------------------------

def build_scan_nc():
    nc = bacc.Bacc("TRN2", target_bir_lowering=False, debug=False,
                   num_devices=NCORES)

    wiT_d = nc.dram_tensor("wiT", [128, 2, 768], BF16, kind="ExternalInput")
    whT_d = nc.dram_tensor("whT", [128, 2, 768], BF16, kind="ExternalInput")
    xT_d = nc.dram_tensor("xT", [128, 2, SB], BF16, kind="ExternalInput")
    xTf_d = nc.dram_tensor("xTf", [128, 2, SB], F32, kind="ExternalInput")
    pT_d = nc.dram_tensor("pT", [128, 2, 3], F32, kind="ExternalInput")
    mask_d = nc.dram_tensor("mask", [128, SB // 128], F32, kind="ExternalInput")
    # bias rows (K=1 matmul stationary operands), bf16
    bri_d = nc.dram_tensor("bri", [1, 512], BF16, kind="ExternalInput")
    bin_d = nc.dram_tensor("bin", [1, 256], BF16, kind="ExternalInput")
    bhn_d = nc.dram_tensor("bhn", [128, 2], F32, kind="ExternalInput")
    hn_out = nc.dram_tensor("hn_out", [128, 2 * BH], BF16, kind="ExternalOutput")
    cf_lin = nc.dram_tensor("cf_lin", [SB // 128, 128], F32)

    NT = SB // 128  # 25 (s,b)-tiles for concen

    with tile.TileContext(nc) as tc:
        with (
            tc.tile_pool(name="consts", bufs=1) as consts,
            tc.tile_pool(name="cw", bufs=1) as cw,
            tc.tile_pool(name="gx", bufs=1, space="PSUM") as gx,
            tc.tile_pool(name="ghn", bufs=1, space="PSUM") as ghnp,
            tc.tile_pool(name="ew", bufs=3) as ew,
            tc.tile_pool(name="hpool", bufs=2) as hpool,
        ):
            wiT = consts.tile_from(wiT_d.ap())
            whT = consts.tile_from(whT_d.ap())
            xT = consts.tile_from(xT_d.ap())
            xTf = consts.tile_from(xTf_d.ap())
            pT = consts.tile_from(pT_d.ap())
            mask = consts.tile_from(mask_d.ap())
            bri = consts.tile_from(bri_d.ap())
            bin_ = consts.tile_from(bin_d.ap())
            bhn = consts.tile_from(bhn_d.ap())
            ones = consts.tile([1, GS * BH], BF16)
            nc.vector.memset(ones[:], 1.0)

            # ---------------- concen -> cf_rep ----------------
            ps_s = ghnp.tile([128, NT, 3], F32, tag="ghn0", name="ps_s")
            for tt in range(NT):
                for k in range(2):
                    nc.tensor.matmul(
                        ps_s[:, tt, :], xTf[:, k, ts(tt, 128)], pT[:, k, :],
                        start=(k == 0), stop=(k == 1))
            e_s = cw.tile([128, NT, 3], F32)
            nc.scalar.activation(e_s[:], ps_s[:], AF.Exp, scale=1.0 / TAU)
            den = cw.tile([128, NT], F32)
            nc.vector.tensor_reduce(den[:], e_s[:], mybir.AxisListType.X, OP.add)
            rden = cw.tile([128, NT], F32)
            nc.vector.reciprocal_approx_fast(rden[:], den[:])
            cnorm = cw.tile([128, NT, 3], F32)
            nc.vector.tensor_tensor(
                cnorm[:], e_s[:],
                rden[:, :, None].to_broadcast((128, NT, 3)), OP.mult)
            cm = cw.tile([128, NT, 3], F32)
            nc.vector.tensor_tensor(
                cm[:], cnorm[:],
                mask[:, :, None].to_broadcast((128, NT, 3)), OP.mult)
            ge = cw.tile([128, NT, 3], F32)
            nc.vector.tensor_scalar(ge[:], cm[:], EPS, None, OP.is_ge)
            cf3 = cw.tile([128, NT, 3], F32)
            nc.vector.tensor_tensor(cf3[:], cm[:], ge[:], OP.mult)
            cf_p = cf3[:, :, 0]  # host permutes purposes: col 0 = this core's

            nc.sync.dma_start(cf_lin.ap().rearrange("t p -> p t"), cf_p)
            cf_rep = cw.tile([128, SB], F32)
            nc.sync.dma_start(
                cf_rep[:],
                cf_lin.ap().rearrange("t p -> (t p)")[None, :]
                .to_broadcast((128, SB)))

            # ---------------- the scan ----------------
            # two independent B=32 sub-scans (batch quarters) interleave so
            # each dependency chain hides in the other's bubbles; recurrent
            # matmuls share LDWEIGHTS between the subs.
            SW = BH // 2  # 32
            h = []
            h_bf = []
            for s_ in range(2):
                hs = hpool.tile([128, 2, SW], F32, tag=f"h{s_}",
                                name=f"h_init{s_}")
                nc.vector.memset(hs[:], 0.0)
                hbs = hpool.tile([128, 2, SW], BF16, tag=f"hbf{s_}",
                                 name=f"hbf_init{s_}")
                nc.vector.memset(hbs[:], 0.0)
                h.append(hs)
                h_bf.append(hbs)

            for g0, gn in GROUPS:
                gw = gn * BH
                # x-side matmuls for the whole group, bias seeded via K=1 mm
                g_ri = gx.tile([128, 4, GS, BH], F32, tag="gri", name="g_ri")
                g_in = gx.tile([128, 2, GS, BH], F32, tag="gin", name="g_in")
                for j in range(4):
                    nc.tensor.matmul(
                        g_ri[:, j, :gn, :], bri[0:1, ts(j, 128)], ones[0:1, :gw],
                        start=True, stop=False)
                    for k in range(2):
                        nc.tensor.matmul(
                            g_ri[:, j, :gn, :], wiT[:, k, ts(j, 128)],
                            xT[:, k, ds(g0 * BH, gw)], start=False,
                            stop=(k == 1))
                for j in range(2):
                    nc.tensor.matmul(
                        g_in[:, j, :gn, :], bin_[0:1, ts(j, 128)], ones[0:1, :gw],
                        start=True, stop=False)
                    for k in range(2):
                        nc.tensor.matmul(
                            g_in[:, j, :gn, :], wiT[:, k, ds(512 + j * 128, 128)],
                            xT[:, k, ds(g0 * BH, gw)], start=False,
                            stop=(k == 1))  # noqa: E501

                for tl in range(gn):
                    t = g0 + tl
                    # interleaved sub-scans: A's block fully precedes B's so
                    # PSUM-bank WAR deps stagger the chains by half a step
                    for s_ in range(2):
                        bsl = ds(s_ * SW, SW)
                        ps_ghn = ghnp.tile([128, 2, SW], F32, tag=f"ghn{s_}",
                                           name=f"ps_ghn{s_}")
                        for j in range(2):
                            for k in range(2):
                                nc.tensor.matmul(
                                    ps_ghn[:, j, :],
                                    whT[:, k, ds(512 + j * 128, 128)],
                                    h_bf[s_][:, k, :], start=(k == 0),
                                    stop=(k == 1))
                        for j in range(4):
                            for k in range(2):
                                nc.tensor.matmul(
                                    g_ri[:, j, tl, bsl],
                                    whT[:, k, ts(j, 128)],
                                    h_bf[s_][:, k, :], start=False, stop=False,
                                    skip_group_check=True)

                        ri_bf = ew.tile([128, 4, SW], BF16, tag=f"ri{s_}",
                                        name=f"ri_bf{s_}")
                        nc.scalar.activation(ri_bf[:], g_ri[:, :, tl, bsl],
                                             AF.Sigmoid)
                        u1 = ew.tile([128, 2, SW], F32, tag=f"u1{s_}",
                                     name=f"u1_{s_}")
                        for j in range(2):
                            nc.vector.scalar_tensor_tensor(
                                u1[:, j, :], ps_ghn[:, j, :], bhn[:, j:j + 1],
                                ri_bf[:, j, :], OP.add, OP.mult)
                        u2 = ew.tile([128, 2, SW], F32, tag=f"u2{s_}",
                                     name=f"u2_{s_}")
                        nc.vector.tensor_tensor(u2[:], u1[:],
                                                g_in[:, :, tl, bsl], OP.add)
                        n_t = ew.tile([128, 2, SW], F32, tag=f"n{s_}",
                                      name=f"n_t{s_}")
                        nc.scalar.activation(n_t[:], u2[:], AF.Tanh)
                        a_t = ew.tile([128, 2, SW], F32, tag=f"a{s_}",
                                      name=f"a_t{s_}")
                        nc.gpsimd.tensor_tensor(
                            a_t[:], ri_bf[:, 2:4, :],
                            cf_rep[:, None, ds(t * BH + s_ * SW, SW)]
                            .to_broadcast((128, 2, SW)), OP.mult)
                        dn = ew.tile([128, 2, SW], F32, tag=f"dn{s_}",
                                     name=f"dn{s_}")
                        nc.vector.tensor_tensor(dn[:], n_t[:], h[s_][:],
                                                OP.subtract)
                        prod = ew.tile([128, 2, SW], F32, tag=f"pr{s_}",
                                       name=f"prod{s_}")
                        nc.vector.tensor_tensor(prod[:], a_t[:], dn[:],
                                                OP.mult)
                        h_new = hpool.tile([128, 2, SW], F32, tag=f"h{s_}",
                                           name=f"h_new{s_}")
                        nc.vector.tensor_tensor(h_new[:], h[s_][:], prod[:],
                                                OP.add)
                        h_bf_new = hpool.tile([128, 2, SW], BF16,
                                              tag=f"hbf{s_}",
                                              name=f"h_bf_new{s_}")
                        if s_ == 0:
                            nc.gpsimd.tensor_copy(h_bf_new[:], h_new[:])
                        else:
                            nc.vector.tensor_copy(h_bf_new[:], h_new[:])
                        h[s_] = h_new
                        h_bf[s_] = h_bf_new

            for s_ in range(2):
                nc.sync.dma_start(
                    hn_out.ap().rearrange("p (k b) -> p k b", k=2)
                    [:, :, ds(s_ * SW, SW)], h_bf[s_][:])

    nc.compile()
    return nc


def scan_host_inputs(seq, emb, emb_purpose, w_ih, w_hh, b_ih, b_hh):
    seq = np.asarray(seq)
    xg = emb[seq]                      # [S, B, D] gather (input staging)
    in_maps = []
    for c in range(NCORES):
        p, h = CORE_PH[c]
        sl = slice(h * BH, (h + 1) * BH)
        xh = xg[:, sl, :]              # [S, BH, D]
        xT = np.ascontiguousarray(
            xh.transpose(2, 0, 1).reshape(2, 128, SB).transpose(1, 0, 2))
        wiT = np.ascontiguousarray(
            w_ih[p].T.reshape(2, 128, 768).transpose(1, 0, 2))
        whT = np.ascontiguousarray(
            w_hh[p].T.reshape(2, 128, 768).transpose(1, 0, 2))
        perm = [p, (p + 1) % 3, (p + 2) % 3]
        pT = np.ascontiguousarray(
            emb_purpose[perm].T.reshape(2, 128, 3).transpose(1, 0, 2))
        m = (seq[:, sl] != 0).astype(np.float32).reshape(SB)
        mask = np.ascontiguousarray(m.reshape(SB // 128, 128).T)
        bsum = (b_ih[p] + b_hh[p])[:512]
        in_maps.append({
            "wiT": wiT.astype(_BF), "whT": whT.astype(_BF),
            "xT": xT.astype(_BF), "xTf": xT.astype(np.float32),
            "pT": pT.astype(np.float32), "mask": mask,
            "bri": bsum[None, :].astype(_BF),
            "bin": b_ih[p][None, 512:].astype(_BF),
            "bhn": np.ascontiguousarray(
                b_hh[p][512:].reshape(2, 128).T).astype(np.float32),
        })
    return in_maps


# --------------------------------------------------------------------------
# Launch 2: scoring
# --------------------------------------------------------------------------

def build_score_nc():
    nc = bacc.Bacc("TRN2", target_bir_lowering=False, debug=False,
                   num_devices=NCORES)

    hT6_d = nc.dram_tensor("hT6", [128, 6, 128], BF16, kind="ExternalInput")
    eT_d = nc.dram_tensor("eT", [128, 2, T_PAD], BF16, kind="ExternalInput")
    pT_d = nc.dram_tensor("pTs", [128, 2, 3], BF16, kind="ExternalInput")
    sel_d = nc.dram_tensor("sel", [128, 4, 128], BF16, kind="ExternalInput")
    scores_d = nc.dram_tensor("scores", [128, T_PAD], F32, kind="ExternalOutput")

    with tile.TileContext(nc) as tc:
        with (
            tc.tile_pool(name="consts", bufs=1) as consts,
            tc.tile_pool(name="spsum", bufs=1, space="PSUM") as spsum,
            tc.tile_pool(name="epsum", bufs=1, space="PSUM") as epsum,
            tc.tile_pool(name="work", bufs=3) as work,
            tc.tile_pool(name="outp", bufs=3) as outp,
        ):
            hT6 = consts.tile_from(hT6_d.ap())
            eT = consts.tile_from(eT_d.ap())
            pT = consts.tile_from(pT_d.ap())
            sel = consts.tile_from(sel_d.ap())

            # exp(logits), rows 0:3 only (K=3 matmuls read just those rows)
            sE = consts.tile([128, T_PAD], BF16)

            for ci, (c0, cs) in enumerate(zip(CHUNK_OFFS[:-1], CHUNK_SIZES)):
                ps_s = spsum.tile([128, N_CHUNK], F32, tag="logit",
                                  name="ps_s")
                for k in range(2):
                    nc.tensor.matmul(
                        ps_s[0:3, :cs], pT[:, k, :], eT[:, k, ds(c0, cs)],
                        start=(k == 0), stop=(k == 1))
                nc.scalar.activation(sE[0:3, ds(c0, cs)], ps_s[0:3, :cs], AF.Exp)

            for ci, (c0, cs) in enumerate(zip(CHUNK_OFFS[:-1], CHUNK_SIZES)):
                ps_P = [epsum.tile([128, N_CHUNK], F32, tag=f"P{p}",
                                   name=f"psP{p}") for p in range(3)]
                for p in range(3):
                    for k in range(2):
                        nc.tensor.matmul(
                            ps_P[p][:, :cs], hT6[:, p * 2 + k, :],
                            eT[:, k, ds(c0, cs)], start=(k == 0), stop=(k == 1))
                ps_E = [epsum.tile([128, N_CHUNK], F32, tag=f"E{j}",
                                   name=f"psE{j}") for j in range(4)]
                for j in range(4):
                    nc.tensor.matmul(
                        ps_E[j][:, :cs], sel[0:3, j, :], sE[0:3, ds(c0, cs)],
                        start=True, stop=True)
                den_f = work.tile([128, N_CHUNK], F32, tag="denf", name="den_f")
                nc.scalar.copy(den_f[:, :cs], ps_E[3][:, :cs])
                rden = work.tile([128, N_CHUNK], F32, tag="rden", name="rden")
                nc.vector.reciprocal_approx_fast(rden[:, :cs], den_f[:, :cs])
                e_sb = [work.tile([128, N_CHUNK], BF16, tag=f"esb{p}",
                                  name=f"esb{p}") for p in range(3)]
                nc.scalar.copy(e_sb[0][:, :cs], ps_E[0][:, :cs])
                nc.scalar.copy(e_sb[1][:, :cs], ps_E[1][:, :cs])
                nc.vector.tensor_copy(e_sb[2][:, :cs], ps_E[2][:, :cs])
                t_p = [work.tile([128, N_CHUNK], BF16, tag=f"tp{p}",
                                 name=f"tp{p}") for p in range(3)]
                for p in range(3):
                    nc.vector.tensor_tensor(
                        t_p[p][:, :cs], ps_P[p][:, :cs], e_sb[p][:, :cs],
                        OP.mult)
                s01 = work.tile([128, N_CHUNK], BF16, tag="s01", name="s01")
                nc.gpsimd.tensor_tensor(
                    s01[:, :cs], t_p[0][:, :cs], t_p[1][:, :cs], OP.add)
                s012 = work.tile([128, N_CHUNK], BF16, tag="s012", name="s012")
                nc.gpsimd.tensor_tensor(
                    s012[:, :cs], s01[:, :cs], t_p[2][:, :cs], OP.add)
                out_c = outp.tile([128, N_CHUNK], F32, tag="out", name="out_c")
                nc.vector.tensor_tensor(
                    out_c[:, :cs], s012[:, :cs], rden[:, :cs], OP.mult)
                nc.sync.dma_start(scores_d.ap()[:, ds(c0, cs)], out_c[:, :cs])

    nc.compile()
    return nc


def score_host_inputs(hn_bf, emb, emb_purpose):
    embT = emb.T.astype(_BF)  # [256, 50001]
    pT = np.ascontiguousarray(
        emb_purpose.T.reshape(2, 128, 3).transpose(1, 0, 2)).astype(_BF)
    sel = np.zeros((128, 4, 128), np.float32)
    for p in range(3):
        sel[p, p, :] = 1.0
        sel[p, 3, :] = 1.0
    sel = sel.astype(_BF)

    base = N_ITEMS // NCORES
    rem = N_ITEMS - base * NCORES
    bounds = []
    s0 = 0
    for c in range(NCORES):
        n = base + (1 if c < rem else 0)
        bounds.append((s0, s0 + n))
        s0 += n

    in_maps = []
    for c in range(NCORES):
        lo, hi = bounds[c]
        n = hi - lo
        eT = np.zeros((128, 2, T_PAD), _BF)
        chunk = embT[:, lo:hi]
        eT[:, :, :n] = chunk.reshape(2, 128, n).transpose(1, 0, 2)
        in_maps.append({"hT6": hn_bf, "eT": eT, "pTs": pT, "sel": sel})
    return in_maps, bounds


# --------------------------------------------------------------------------
# Entry point
# --------------------------------------------------------------------------

_SCAN_NC = None
_SCORE_NC = None


def _get_ncs():
    global _SCAN_NC, _SCORE_NC
    if _SCAN_NC is None:
        _SCAN_NC = build_scan_nc()
    if _SCORE_NC is None:
        _SCORE_NC = build_score_nc()
    return _SCAN_NC, _SCORE_NC


def kernel(seq, emb, emb_purpose, w_ih, w_hh, b_ih, b_hh):
    seq = np.asarray(seq)
    emb = np.asarray(emb, np.float32)
    emb_purpose = np.asarray(emb_purpose, np.float32)
    w_ih = np.asarray(w_ih, np.float32)
    w_hh = np.asarray(w_hh, np.float32)
    b_ih = np.asarray(b_ih, np.float32)
    b_hh = np.asarray(b_hh, np.float32)

    scan_nc, score_nc = _get_ncs()

    scan_ins = scan_host_inputs(seq, emb, emb_purpose, w_ih, w_hh, b_ih, b_hh)
    res1 = run_bass_kernel_spmd(scan_nc, scan_ins, core_ids=list(range(NCORES)))

    hT6 = np.zeros((128, 6, 128), _BF)
    for c in range(6):
        p, h = CORE_PH[c]
        sl = res1.results[c]["hn_out"].reshape(128, 2, BH)
        for k in range(2):
            hT6[:, p * 2 + k, h * BH:(h + 1) * BH] = sl[:, k, :]

    score_ins, bounds = score_host_inputs(hT6, emb, emb_purpose)
    res2 = run_bass_kernel_spmd(score_nc, score_ins, core_ids=list(range(NCORES)))

    scores = np.empty((B, N_ITEMS), np.float32)
    for c in range(NCORES):
        lo, hi = bounds[c]
        scores[:, lo:hi] = res2.results[c]["scores"][:, : hi - lo]
    return scores


# revision 16
# speedup vs baseline: 1.1694x; 1.0620x over previous
"""Trainium2 Bass kernel for MCPRN (purpose-routed GRU-variant session recommender).

Pipeline (two SPMD launches on 8 NeuronCores):
  Launch 1 (scan): cores run (purpose p, batch-half h) PSRU scans, B_local=64.
     6 real slots + 2 duplicates. bf16 matmuls, fp32 elementwise state.
     Biases enter PSUM via K=1 ones-matmuls; the x-side (input) matmuls are
     batched 4 steps at a time (N=256) and the recurrent matmuls accumulate
     into the same PSUM group. Concentration weights (softmax over purposes
     of x . emb_purpose / tau, masked, eps-clamped) are computed on device in
     fp32 and broadcast across partitions via a DRAM roundtrip.
  Host gathers final hidden states hn[3, 128, 256] (bf16).
  Launch 2 (score): cores each score an item chunk (~6250 of 50001 items):
     scores[b, t] = sum_p tcw[t, p] * <hn[p, b, :], emb[t, :]>,
     tcw = softmax_p(emb @ emb_purpose.T). Softmax weights are broadcast
     across partitions with ones-matmuls; combine on DVE/GPSIMD.
"""

import numpy as np
import ml_dtypes

import concourse.bacc as bacc
import concourse.mybir as mybir
import concourse.tile as tile
from concourse.bass import ts, ds
from concourse.bass_utils import run_bass_kernel_spmd

F32 = mybir.dt.float32
BF16 = mybir.dt.bfloat16
AF = mybir.ActivationFunctionType
OP = mybir.AluOpType

N_ITEMS = 50001
DIM = 256
TAU = 0.1
S = 50
B = 128
EPS = 0.01
BH = 64         # batch half per scan core
SB = S * BH     # 3200 (step, batch) elements per scan core
NCORES = 8
GS = 8          # steps per x-side matmul group (8*64 = 512 f32 = 1 PSUM bank)
GROUPS = [(g, min(GS, S - g)) for g in range(0, S, GS)]

# scoring chunking
T_PAD = 6272            # 49 * 128, per-core padded item count
N_CHUNK = 512
CHUNK_SIZES = [512] * 12 + [128]
CHUNK_OFFS = np.cumsum([0] + CHUNK_SIZES).tolist()

CORE_PH = [(0, 0), (0, 1), (1, 0), (1, 1), (2, 0), (2, 1), (0, 0), (0, 1)]

_BF = ml_dtypes.bfloat16


# --------------------------------------------------------------------------
# Launch 1: scan
# --------------------------------------------------------------------------

def build_scan_nc():
    nc = bacc.Bacc("TRN2", target_bir_lowering=False, debug=False,
                   num_devices=NCORES)

    wiT_d = nc.dram_tensor("wiT", [128, 2, 768], BF16, kind="ExternalInput")
    whT_d = nc.dram_tensor("whT", [128, 2, 768], BF16, kind="ExternalInput")
    xT_d = nc.dram_tensor("xT", [128, 2, SB], BF16, kind="ExternalInput")
    xTf_d = nc.dram_tensor("xTf", [128, 2, SB], F32, kind="ExternalInput")
    pT_d = nc.dram_tensor("pT", [128, 2, 3], F32, kind="ExternalInput")
    mask_d = nc.dram_tensor("mask", [128, SB // 128], F32, kind="ExternalInput")
    # bias rows (K=1 matmul stationary operands), bf16
    bri_d = nc.dram_tensor("bri", [1, 512], BF16, kind="ExternalInput")
    bin_d = nc.dram_tensor("bin", [1, 256], BF16, kind="ExternalInput")
    bhn_d = nc.dram_tensor("bhn", [128, 2], F32, kind="ExternalInput")
    hn_out = nc.dram_tensor("hn_out", [128, 2 * BH], BF16, kind="ExternalOutput")
    cf_lin = nc.dram_tensor("cf_lin", [SB // 128, 128], F32)

    NT = SB // 128  # 25 (s,b)-tiles for concen

    with tile.TileContext(nc) as tc:
        with (
            tc.tile_pool(name="consts", bufs=1) as consts,
            tc.tile_pool(name="cw", bufs=1) as cw,
            tc.tile_pool(name="gx", bufs=1, space="PSUM") as gx,
            tc.tile_pool(name="ghn", bufs=1, space="PSUM") as ghnp,
            tc.tile_pool(name="ew", bufs=3) as ew,
            tc.tile_pool(name="hpool", bufs=2) as hpool,
        ):
            wiT = consts.tile_from(wiT_d.ap())
            whT = consts.tile_from(whT_d.ap())
            xT = consts.tile_from(xT_d.ap())
            xTf = consts.tile_from(xTf_d.ap())
            pT = consts.tile_from(pT_d.ap())
            mask = consts.tile_from(mask_d.ap())
            bri = consts.tile_from(bri_d.ap())
            bin_ = consts.tile_from(bin_d.ap())
            bhn = consts.tile_from(bhn_d.ap())
            ones = consts.tile([1, GS * BH], BF16)
            nc.vector.memset(ones[:], 1.0)

            # ---------------- concen -> cf_rep ----------------
            ps_s = ghnp.tile([128, NT, 3], F32, tag="ghn0", name="ps_s")
            for tt in range(NT):
                for k in range(2):
                    nc.tensor.matmul(
                        ps_s[:, tt, :], xTf[:, k, ts(tt, 128)], pT[:, k, :],
                        start=(k == 0), stop=(k == 1))
            e_s = cw.tile([128, NT, 3], F32)
            nc.scalar.activation(e_s[:], ps_s[:], AF.Exp, scale=1.0 / TAU)
            den = cw.tile([128, NT], F32)
            nc.vector.tensor_reduce(den[:], e_s[:], mybir.AxisListType.X, OP.add)
            rden = cw.tile([128, NT], F32)
            nc.vector.reciprocal_approx_fast(rden[:], den[:])
            cnorm = cw.tile([128, NT, 3], F32)
            nc.vector.tensor_tensor(
                cnorm[:], e_s[:],
                rden[:, :, None].to_broadcast((128, NT, 3)), OP.mult)
            cm = cw.tile([128, NT, 3], F32)
            nc.vector.tensor_tensor(
                cm[:], cnorm[:],
                mask[:, :, None].to_broadcast((128, NT, 3)), OP.mult)
            ge = cw.tile([128, NT, 3], F32)
            nc.vector.tensor_scalar(ge[:], cm[:], EPS, None, OP.is_ge)
            cf3 = cw.tile([128, NT, 3], F32)
            nc.vector.tensor_tensor(cf3[:], cm[:], ge[:], OP.mult)
            cf_p = cf3[:, :, 0]  # host permutes purposes: col 0 = this core's

            nc.sync.dma_start(cf_lin.ap().rearrange("t p -> p t"), cf_p)
            cf_rep = cw.tile([128, SB], F32)
            nc.sync.dma_start(
                cf_rep[:],
                cf_lin.ap().rearrange("t p -> (t p)")[None, :]
                .to_broadcast((128, SB)))

            # ---------------- the scan ----------------
            # two independent B=32 sub-scans (batch quarters) interleave so
            # each dependency chain hides in the other's bubbles; recurrent
            # matmuls share LDWEIGHTS between the subs.
            SW = BH // 2  # 32
            h = []
            h_bf = []
            for s_ in range(2):
                hs = hpool.tile([128, 2, SW], F32, tag=f"h{s_}",
                                name=f"h_init{s_}")
                nc.vector.memset(hs[:], 0.0)
                hbs = hpool.tile([128, 2, SW], BF16, tag=f"hbf{s_}",
                                 name=f"hbf_init{s_}")
                nc.vector.memset(hbs[:], 0.0)
                h.append(hs)
                h_bf.append(hbs)

            for g0, gn in GROUPS:
                gw = gn * BH
                # x-side matmuls for the whole group, bias seeded via K=1 mm
                g_ri = gx.tile([128, 4, GS, BH], F32, tag="gri", name="g_ri")
                g_in = gx.tile([128, 2, GS, BH], F32, tag="gin", name="g_in")
                for j in range(4):
                    nc.tensor.matmul(
                        g_ri[:, j, :gn, :], bri[0:1, ts(j, 128)], ones[0:1, :gw],
                        start=True, stop=False)
                    for k in range(2):
                        nc.tensor.matmul(
                            g_ri[:, j, :gn, :], wiT[:, k, ts(j, 128)],
                            xT[:, k, ds(g0 * BH, gw)], start=False,
                            stop=(k == 1))
                for j in range(2):
                    nc.tensor.matmul(
                        g_in[:, j, :gn, :], bin_[0:1, ts(j, 128)], ones[0:1, :gw],
                        start=True, stop=False)
                    for k in range(2):
                        nc.tensor.matmul(
                            g_in[:, j, :gn, :], wiT[:, k, ds(512 + j * 128, 128)],
                            xT[:, k, ds(g0 * BH, gw)], start=False,
                            stop=(k == 1))  # noqa: E501

                for tl in range(gn):
                    t = g0 + tl
                    # interleaved sub-scans: A's block fully precedes B's so
                    # PSUM-bank WAR deps stagger the chains by half a step
                    for s_ in range(2):
                        bsl = ds(s_ * SW, SW)
                        ps_ghn = ghnp.tile([128, 2, SW], F32, tag=f"ghn{s_}",
                                           name=f"ps_ghn{s_}")
                        for j in range(2):
                            for k in range(2):
                                nc.tensor.matmul(
                                    ps_ghn[:, j, :],
                                    whT[:, k, ds(512 + j * 128, 128)],
                                    h_bf[s_][:, k, :], start=(k == 0),
                                    stop=(k == 1))
                        for j in range(4):
                            for k in range(2):
                                nc.tensor.matmul(
                                    g_ri[:, j, tl, bsl],
                                    whT[:, k, ts(j, 128)],
                                    h_bf[s_][:, k, :], start=False, stop=False,
                                    skip_group_check=True)

                        ri_bf = ew.tile([128, 4, SW], BF16, tag=f"ri{s_}",
                                        name=f"ri_bf{s_}")
                        nc.scalar.activation(ri_bf[:], g_ri[:, :, tl, bsl],
                                             AF.Sigmoid)
                        u1 = ew.tile([128, 2, SW], F32, tag=f"u1{s_}",
                                     name=f"u1_{s_}")
                        for j in range(2):
                            nc.vector.scalar_tensor_tensor(
                                u1[:, j, :], ps_ghn[:, j, :], bhn[:, j:j + 1],
                                ri_bf[:, j, :], OP.add, OP.mult)
                        u2 = ew.tile([128, 2, SW], F32, tag=f"u2{s_}",
                                     name=f"u2_{s_}")
                        nc.vector.tensor_tensor(u2[:], u1[:],
                                                g_in[:, :, tl, bsl], OP.add)
                        n_t = ew.tile([128, 2, SW], F32, tag=f"n{s_}",
                                      name=f"n_t{s_}")
                        nc.scalar.activation(n_t[:], u2[:], AF.Tanh)
                        a_t = ew.tile([128, 2, SW], F32, tag=f"a{s_}",
                                      name=f"a_t{s_}")
                        nc.gpsimd.tensor_tensor(
                            a_t[:], ri_bf[:, 2:4, :],
                            cf_rep[:, None, ds(t * BH + s_ * SW, SW)]
                            .to_broadcast((128, 2, SW)), OP.mult)
                        q_t = ew.tile([128, 2, SW], F32, tag=f"q{s_}",
                                      name=f"q_t{s_}")
                        nc.vector.tensor_scalar(q_t[:], a_t[:], -1.0, 1.0,
                                                OP.mult, OP.add)
                        hq = ew.tile([128, 2, SW], F32, tag=f"hqt{s_}",
                                     name=f"hq{s_}")
                        nc.gpsimd.tensor_tensor(hq[:], h[s_][:], q_t[:],
                                                OP.mult)
                        an = ew.tile([128, 2, SW], F32, tag=f"ant{s_}",
                                     name=f"an{s_}")
                        nc.vector.tensor_tensor(an[:], a_t[:], n_t[:], OP.mult)
                        h_new = hpool.tile([128, 2, SW], F32, tag=f"h{s_}",
                                           name=f"h_new{s_}")
                        nc.vector.tensor_tensor(h_new[:], hq[:], an[:],
                                                OP.add)
                        h_bf_new = hpool.tile([128, 2, SW], BF16,
                                              tag=f"hbf{s_}",
                                              name=f"h_bf_new{s_}")
                        if s_ == 0:
                            nc.gpsimd.tensor_copy(h_bf_new[:], h_new[:])
                        else:
                            nc.vector.tensor_copy(h_bf_new[:], h_new[:])
                        h[s_] = h_new
                        h_bf[s_] = h_bf_new

            for s_ in range(2):
                nc.sync.dma_start(
                    hn_out.ap().rearrange("p (k b) -> p k b", k=2)
                    [:, :, ds(s_ * SW, SW)], h_bf[s_][:])

    nc.compile()
    return nc


def scan_host_inputs(seq, emb, emb_purpose, w_ih, w_hh, b_ih, b_hh):
    seq = np.asarray(seq)
    xg = emb[seq]                      # [S, B, D] gather (input staging)
    in_maps = []
    for c in range(NCORES):
        p, h = CORE_PH[c]
        sl = slice(h * BH, (h + 1) * BH)
        xh = xg[:, sl, :]              # [S, BH, D]
        xT = np.ascontiguousarray(
            xh.transpose(2, 0, 1).reshape(2, 128, SB).transpose(1, 0, 2))
        wiT = np.ascontiguousarray(
            w_ih[p].T.reshape(2, 128, 768).transpose(1, 0, 2))
        whT = np.ascontiguousarray(
            w_hh[p].T.reshape(2, 128, 768).transpose(1, 0, 2))
        perm = [p, (p + 1) % 3, (p + 2) % 3]
        pT = np.ascontiguousarray(
            emb_purpose[perm].T.reshape(2, 128, 3).transpose(1, 0, 2))
        m = (seq[:, sl] != 0).astype(np.float32).reshape(SB)
        mask = np.ascontiguousarray(m.reshape(SB // 128, 128).T)
        bsum = (b_ih[p] + b_hh[p])[:512]
        in_maps.append({
            "wiT": wiT.astype(_BF), "whT": whT.astype(_BF),
            "xT": xT.astype(_BF), "xTf": xT.astype(np.float32),
            "pT": pT.astype(np.float32), "mask": mask,
            "bri": bsum[None, :].astype(_BF),
            "bin": b_ih[p][None, 512:].astype(_BF),
            "bhn": np.ascontiguousarray(
                b_hh[p][512:].reshape(2, 128).T).astype(np.float32),
        })
    return in_maps


# --------------------------------------------------------------------------
# Launch 2: scoring
# --------------------------------------------------------------------------

def build_score_nc():
    nc = bacc.Bacc("TRN2", target_bir_lowering=False, debug=False,
                   num_devices=NCORES)

    hT6_d = nc.dram_tensor("hT6", [128, 6, 128], BF16, kind="ExternalInput")
    eT_d = nc.dram_tensor("eT", [128, 2, T_PAD], BF16, kind="ExternalInput")
    pT_d = nc.dram_tensor("pTs", [128, 2, 3], BF16, kind="ExternalInput")
    sel_d = nc.dram_tensor("sel", [128, 4, 128], BF16, kind="ExternalInput")
    scores_d = nc.dram_tensor("scores", [128, T_PAD], F32, kind="ExternalOutput")

    with tile.TileContext(nc) as tc:
        with (
            tc.tile_pool(name="consts", bufs=1) as consts,
            tc.tile_pool(name="spsum", bufs=1, space="PSUM") as spsum,
            tc.tile_pool(name="epsum", bufs=1, space="PSUM") as epsum,
            tc.tile_pool(name="work", bufs=4) as work,
            tc.tile_pool(name="outp", bufs=4) as outp,
        ):
            hT6 = consts.tile_from(hT6_d.ap())
            eT = consts.tile_from(eT_d.ap())
            pT = consts.tile_from(pT_d.ap())
            sel = consts.tile_from(sel_d.ap())

            # exp(logits), rows 0:3 only (K=3 matmuls read just those rows)
            sE = consts.tile([128, T_PAD], BF16)

            for ci, (c0, cs) in enumerate(zip(CHUNK_OFFS[:-1], CHUNK_SIZES)):
                ps_s = spsum.tile([128, N_CHUNK], F32, tag="logit",
                                  name="ps_s")
                for k in range(2):
                    nc.tensor.matmul(
                        ps_s[0:3, :cs], pT[:, k, :], eT[:, k, ds(c0, cs)],
                        start=(k == 0), stop=(k == 1))
                nc.scalar.activation(sE[0:3, ds(c0, cs)], ps_s[0:3, :cs], AF.Exp)

            for ci, (c0, cs) in enumerate(zip(CHUNK_OFFS[:-1], CHUNK_SIZES)):
                ps_P = [epsum.tile([128, N_CHUNK], F32, tag=f"P{p}",
                                   name=f"psP{p}") for p in range(3)]
                for p in range(3):
                    for k in range(2):
                        nc.tensor.matmul(
                            ps_P[p][:, :cs], hT6[:, p * 2 + k, :],
                            eT[:, k, ds(c0, cs)], start=(k == 0), stop=(k == 1))
                ps_E = [epsum.tile([128, N_CHUNK], F32, tag=f"E{j}",
                                   name=f"psE{j}") for j in range(4)]
                for j in range(4):
                    nc.tensor.matmul(
                        ps_E[j][:, :cs], sel[0:3, j, :], sE[0:3, ds(c0, cs)],
                        start=True, stop=True)
                den_f = work.tile([128, N_CHUNK], F32, tag="denf", name="den_f")
                nc.scalar.copy(den_f[:, :cs], ps_E[3][:, :cs])
                rden = work.tile([128, N_CHUNK], F32, tag="rden", name="rden")
                nc.vector.reciprocal_approx_fast(rden[:, :cs], den_f[:, :cs])
                e_sb = [work.tile([128, N_CHUNK], BF16, tag=f"esb{p}",
                                  name=f"esb{p}") for p in range(3)]
                nc.scalar.copy(e_sb[0][:, :cs], ps_E[0][:, :cs])
                nc.scalar.copy(e_sb[1][:, :cs], ps_E[1][:, :cs])
                nc.vector.tensor_copy(e_sb[2][:, :cs], ps_E[2][:, :cs])
                t_p = [work.tile([128, N_CHUNK], BF16, tag=f"tp{p}",
                                 name=f"tp{p}") for p in range(3)]
                for p in range(3):
                    nc.vector.tensor_tensor(
                        t_p[p][:, :cs], ps_P[p][:, :cs], e_sb[p][:, :cs],
                        OP.mult)
                s01 = work.tile([128, N_CHUNK], BF16, tag="s01", name="s01")
                nc.gpsimd.tensor_tensor(
                    s01[:, :cs], t_p[0][:, :cs], t_p[1][:, :cs], OP.add)
                s012 = work.tile([128, N_CHUNK], BF16, tag="s012", name="s012")
                nc.gpsimd.tensor_tensor(
                    s012[:, :cs], s01[:, :cs], t_p[2][:, :cs], OP.add)
                out_c = outp.tile([128, N_CHUNK], F32, tag="out", name="out_c")
                nc.gpsimd.tensor_tensor(
                    out_c[:, :cs], s012[:, :cs], rden[:, :cs], OP.mult)
                nc.sync.dma_start(scores_d.ap()[:, ds(c0, cs)], out_c[:, :cs])

    nc.compile()
    return nc


def score_host_inputs(hn_bf, emb, emb_purpose):
    embT = emb.T.astype(_BF)  # [256, 50001]
    pT = np.ascontiguousarray(
        emb_purpose.T.reshape(2, 128, 3).transpose(1, 0, 2)).astype(_BF)
    sel = np.zeros((128, 4, 128), np.float32)
    for p in range(3):
        sel[p, p, :] = 1.0
        sel[p, 3, :] = 1.0
    sel = sel.astype(_BF)

    base = N_ITEMS // NCORES
    rem = N_ITEMS - base * NCORES
    bounds = []
    s0 = 0
    for c in range(NCORES):
        n = base + (1 if c < rem else 0)
        bounds.append((s0, s0 + n))
        s0 += n

    in_maps = []
    for c in range(NCORES):
        lo, hi = bounds[c]
        n = hi - lo
        eT = np.zeros((128, 2, T_PAD), _BF)
        chunk = embT[:, lo:hi]
        eT[:, :, :n] = chunk.reshape(2, 128, n).transpose(1, 0, 2)
        in_maps.append({"hT6": hn_bf, "eT": eT, "pTs": pT, "sel": sel})
    return in_maps, bounds


# --------------------------------------------------------------------------
# Entry point
# --------------------------------------------------------------------------

_SCAN_NC = None
_SCORE_NC = None


def _get_ncs():
    global _SCAN_NC, _SCORE_NC
    if _SCAN_NC is None:
        _SCAN_NC = build_scan_nc()
    if _SCORE_NC is None:
        _SCORE_NC = build_score_nc()
    return _SCAN_NC, _SCORE_NC


def kernel(seq, emb, emb_purpose, w_ih, w_hh, b_ih, b_hh):
    seq = np.asarray(seq)
    emb = np.asarray(emb, np.float32)
    emb_purpose = np.asarray(emb_purpose, np.float32)
    w_ih = np.asarray(w_ih, np.float32)
    w_hh = np.asarray(w_hh, np.float32)
    b_ih = np.asarray(b_ih, np.float32)
    b_hh = np.asarray(b_hh, np.float32)

    scan_nc, score_nc = _get_ncs()

    scan_ins = scan_host_inputs(seq, emb, emb_purpose, w_ih, w_hh, b_ih, b_hh)
    res1 = run_bass_kernel_spmd(scan_nc, scan_ins, core_ids=list(range(NCORES)))

    hT6 = np.zeros((128, 6, 128), _BF)
    for c in range(6):
        p, h = CORE_PH[c]
        sl = res1.results[c]["hn_out"].reshape(128, 2, BH)
        for k in range(2):
            hT6[:, p * 2 + k, h * BH:(h + 1) * BH] = sl[:, k, :]

    score_ins, bounds = score_host_inputs(hT6, emb, emb_purpose)
    res2 = run_bass_kernel_spmd(score_nc, score_ins, core_ids=list(range(NCORES)))

    scores = np.empty((B, N_ITEMS), np.float32)
    for c in range(NCORES):
        lo, hi = bounds[c]
        scores[:, lo:hi] = res2.results[c]["scores"][:, : hi - lo]
    return scores


# revision 23
# speedup vs baseline: 1.2340x; 1.0552x over previous
"""Trainium2 Bass kernel for MCPRN (purpose-routed GRU-variant session recommender).

Pipeline (two SPMD launches on 8 NeuronCores):
  Launch 1 (scan): cores run (purpose p, batch-half h) PSRU scans, B_local=64.
     6 real slots + 2 duplicates. bf16 matmuls, fp32 elementwise state.
     Biases enter PSUM via K=1 ones-matmuls; the x-side (input) matmuls are
     batched 4 steps at a time (N=256) and the recurrent matmuls accumulate
     into the same PSUM group. Concentration weights (softmax over purposes
     of x . emb_purpose / tau, masked, eps-clamped) are computed on device in
     fp32 and broadcast across partitions via a DRAM roundtrip.
  Host gathers final hidden states hn[3, 128, 256] (bf16).
  Launch 2 (score): cores each score an item chunk (~6250 of 50001 items):
     scores[b, t] = sum_p tcw[t, p] * <hn[p, b, :], emb[t, :]>,
     tcw = softmax_p(emb @ emb_purpose.T). Softmax weights are broadcast
     across partitions with ones-matmuls; combine on DVE/GPSIMD.
"""

import numpy as np
import ml_dtypes

import concourse.bacc as bacc
import concourse.mybir as mybir
import concourse.tile as tile
from concourse.bass import ts, ds
from concourse.bass_utils import run_bass_kernel_spmd

F32 = mybir.dt.float32
BF16 = mybir.dt.bfloat16
AF = mybir.ActivationFunctionType
OP = mybir.AluOpType

N_ITEMS = 50001
DIM = 256
TAU = 0.1
S = 50
B = 128
EPS = 0.01
BH = 64         # batch half per scan core
SB = S * BH     # 3200 (step, batch) elements per scan core
NCORES = 8
GS = 8          # steps per x-side matmul group (8*64 = 512 f32 = 1 PSUM bank)
GROUPS = [(g, min(GS, S - g)) for g in range(0, S, GS)]

# scoring chunking
T_PAD = 6272            # 49 * 128, per-core padded item count
N_CHUNK = 512
CHUNK_SIZES = [512] * 12 + [128]
CHUNK_OFFS = np.cumsum([0] + CHUNK_SIZES).tolist()

CORE_PH = [(0, 0), (0, 1), (1, 0), (1, 1), (2, 0), (2, 1), (0, 0), (0, 1)]

_BF = ml_dtypes.bfloat16


# --------------------------------------------------------------------------
# Launch 1: scan
# --------------------------------------------------------------------------

def build_scan_nc():
    nc = bacc.Bacc("TRN2", target_bir_lowering=False, debug=False,
                   num_devices=NCORES)

    wiT_d = nc.dram_tensor("wiT", [128, 2, 768], BF16, kind="ExternalInput")
    whT_d = nc.dram_tensor("whT", [128, 2, 768], BF16, kind="ExternalInput")
    xT_d = nc.dram_tensor("xT", [128, 2, SB], BF16, kind="ExternalInput")
    pT_d = nc.dram_tensor("pT", [128, 2, 3], BF16, kind="ExternalInput")
    mask_d = nc.dram_tensor("mask", [128, SB // 128], F32, kind="ExternalInput")
    # bias rows (K=1 matmul stationary operands), bf16
    bri_d = nc.dram_tensor("bri", [1, 512], BF16, kind="ExternalInput")
    bin_d = nc.dram_tensor("bin", [1, 256], BF16, kind="ExternalInput")
    bhn_d = nc.dram_tensor("bhn", [128, 2], F32, kind="ExternalInput")
    hn_out = nc.dram_tensor("hn_out", [128, 2 * BH], BF16, kind="ExternalOutput")
    cf_lin = nc.dram_tensor("cf_lin", [SB // 128, 128], BF16)

    NT = SB // 128  # 25 (s,b)-tiles for concen

    with tile.TileContext(nc) as tc:
        with (
            tc.tile_pool(name="consts", bufs=1) as consts,
            tc.tile_pool(name="cw", bufs=1) as cw,
            tc.tile_pool(name="gx", bufs=1, space="PSUM") as gx,
            tc.tile_pool(name="ghn", bufs=1, space="PSUM") as ghnp,
            tc.tile_pool(name="ew", bufs=4) as ew,
            tc.tile_pool(name="hpool", bufs=3) as hpool,
        ):
            pT = consts.tile_from(pT_d.ap())
            xT = consts.tile_from(xT_d.ap())
            wiT = consts.tile_from(wiT_d.ap())
            whT = consts.tile_from(whT_d.ap())
            mask = consts.tile_from(mask_d.ap())
            bri = consts.tile_from(bri_d.ap())
            bin_ = consts.tile_from(bin_d.ap())
            bhn = consts.tile_from(bhn_d.ap())
            ones = consts.tile([1, GS * BH], BF16)
            nc.vector.memset(ones[:], 1.0)

            # ---------------- concen -> cf_rep ----------------
            ps_s = ghnp.tile([128, NT, 3], F32, tag="ghn0", name="ps_s")
            for tt in range(NT):
                for k in range(2):
                    nc.tensor.matmul(
                        ps_s[:, tt, :], xT[:, k, ts(tt, 128)], pT[:, k, :],
                        start=(k == 0), stop=(k == 1))
            e_s = cw.tile([128, NT, 3], F32)
            nc.scalar.activation(e_s[:], ps_s[:], AF.Exp, scale=1.0 / TAU)
            den = cw.tile([128, NT], F32)
            nc.vector.tensor_reduce(den[:], e_s[:], mybir.AxisListType.X, OP.add)
            rden = cw.tile([128, NT], F32)
            nc.vector.reciprocal_approx_fast(rden[:], den[:])
            cnorm = cw.tile([128, NT, 3], F32)
            nc.vector.tensor_tensor(
                cnorm[:], e_s[:],
                rden[:, :, None].to_broadcast((128, NT, 3)), OP.mult)
            cm = cw.tile([128, NT, 3], F32)
            nc.vector.tensor_tensor(
                cm[:], cnorm[:],
                mask[:, :, None].to_broadcast((128, NT, 3)), OP.mult)
            ge = cw.tile([128, NT, 3], F32)
            nc.vector.tensor_scalar(ge[:], cm[:], EPS, None, OP.is_ge)
            cf3 = cw.tile([128, NT, 3], BF16)
            nc.vector.tensor_tensor(cf3[:], cm[:], ge[:], OP.mult)
            cf_p = cf3[:, :, 0]  # host permutes purposes: col 0 = this core's

            nc.sync.dma_start(cf_lin.ap().rearrange("t p -> p t"), cf_p)
            cf_rep = cw.tile([128, SB], BF16)
            nc.sync.dma_start(
                cf_rep[:],
                cf_lin.ap().rearrange("t p -> (t p)")[None, :]
                .to_broadcast((128, SB)))

            # ---------------- the scan ----------------
            # two independent B=32 sub-scans (batch quarters) interleave so
            # each dependency chain hides in the other's bubbles; recurrent
            # matmuls share LDWEIGHTS between the subs.
            SW = BH // 2  # 32
            h = []
            h_bf = []
            for s_ in range(2):
                hs = hpool.tile([128, 2, SW], F32, tag=f"h{s_}",
                                name=f"h_init{s_}")
                nc.vector.memset(hs[:], 0.0)
                hbs = hpool.tile([128, 2, SW], BF16, tag=f"hbf{s_}",
                                 name=f"hbf_init{s_}")
                nc.vector.memset(hbs[:], 0.0)
                h.append(hs)
                h_bf.append(hbs)

            for g0, gn in GROUPS:
                gw = gn * BH
                # x-side matmuls for the whole group, bias seeded via K=1 mm
                g_ri = gx.tile([128, 4, GS, BH], F32, tag="gri", name="g_ri")
                g_in = gx.tile([128, 2, GS, BH], F32, tag="gin", name="g_in")
                for j in range(4):
                    nc.tensor.matmul(
                        g_ri[:, j, :gn, :], bri[0:1, ts(j, 128)], ones[0:1, :gw],
                        start=True, stop=False)
                    for k in range(2):
                        nc.tensor.matmul(
                            g_ri[:, j, :gn, :], wiT[:, k, ts(j, 128)],
                            xT[:, k, ds(g0 * BH, gw)], start=False,
                            stop=(k == 1))
                for j in range(2):
                    nc.tensor.matmul(
                        g_in[:, j, :gn, :], bin_[0:1, ts(j, 128)], ones[0:1, :gw],
                        start=True, stop=False)
                    for k in range(2):
                        nc.tensor.matmul(
                            g_in[:, j, :gn, :], wiT[:, k, ds(512 + j * 128, 128)],
                            xT[:, k, ds(g0 * BH, gw)], start=False,
                            stop=(k == 1))  # noqa: E501

                for tl in range(gn):
                    t = g0 + tl
                    # interleaved sub-scans: A's block fully precedes B's so
                    # PSUM-bank WAR deps stagger the chains by half a step
                    for s_ in range(2):
                        bsl = ds(s_ * SW, SW)
                        ps_ghn = ghnp.tile([128, 2, SW], F32, tag=f"ghn{s_}",
                                           name=f"ps_ghn{s_}")
                        for j in range(4):
                            for k in range(2):
                                nc.tensor.matmul(
                                    g_ri[:, j, tl, bsl],
                                    whT[:, k, ts(j, 128)],
                                    h_bf[s_][:, k, :], start=False, stop=False,
                                    skip_group_check=True)
                        for j in range(2):
                            for k in range(2):
                                nc.tensor.matmul(
                                    ps_ghn[:, j, :],
                                    whT[:, k, ds(512 + j * 128, 128)],
                                    h_bf[s_][:, k, :], start=(k == 0),
                                    stop=(k == 1))

                        ri_bf = ew.tile([128, 4, SW], BF16, tag=f"ri{s_}",
                                        name=f"ri_bf{s_}")
                        nc.scalar.activation(ri_bf[:], g_ri[:, :, tl, bsl],
                                             AF.Sigmoid)
                        u1 = ew.tile([128, 2, SW], F32, tag=f"u1{s_}",
                                     name=f"u1_{s_}")
                        for j in range(2):
                            nc.vector.scalar_tensor_tensor(
                                u1[:, j, :], ps_ghn[:, j, :], bhn[:, j:j + 1],
                                ri_bf[:, j, :], OP.add, OP.mult)
                        u2 = ew.tile([128, 2, SW], F32, tag=f"u2{s_}",
                                     name=f"u2_{s_}")
                        nc.vector.tensor_tensor(u2[:], u1[:],
                                                g_in[:, :, tl, bsl], OP.add)
                        n_t = ew.tile([128, 2, SW], F32, tag=f"n{s_}",
                                      name=f"n_t{s_}")
                        nc.scalar.activation(n_t[:], u2[:], AF.Tanh)
                        a_t = ew.tile([128, 2, SW], F32, tag=f"a{s_}",
                                      name=f"a_t{s_}")
                        nc.gpsimd.tensor_tensor(
                            a_t[:], ri_bf[:, 2:4, :],
                            cf_rep[:, None, ds(t * BH + s_ * SW, SW)]
                            .to_broadcast((128, 2, SW)), OP.mult)
                        q_t = ew.tile([128, 2, SW], F32, tag=f"q{s_}",
                                      name=f"q_t{s_}")
                        nc.vector.tensor_scalar(q_t[:], a_t[:], -1.0, 1.0,
                                                OP.mult, OP.add)
                        hq = ew.tile([128, 2, SW], F32, tag=f"hqt{s_}",
                                     name=f"hq{s_}")
                        nc.gpsimd.tensor_tensor(hq[:], h[s_][:], q_t[:],
                                                OP.mult)
                        an = ew.tile([128, 2, SW], F32, tag=f"ant{s_}",
                                     name=f"an{s_}")
                        nc.vector.tensor_tensor(an[:], a_t[:], n_t[:], OP.mult)
                        h_new = hpool.tile([128, 2, SW], F32, tag=f"h{s_}",
                                           name=f"h_new{s_}")
                        nc.vector.tensor_tensor(h_new[:], hq[:], an[:],
                                                OP.add)
                        h_bf_new = hpool.tile([128, 2, SW], BF16,
                                              tag=f"hbf{s_}",
                                              name=f"h_bf_new{s_}")
                        if s_ == 0:
                            nc.gpsimd.tensor_copy(h_bf_new[:], h_new[:])
                        else:
                            nc.vector.tensor_copy(h_bf_new[:], h_new[:])
                        h[s_] = h_new
                        h_bf[s_] = h_bf_new

            for s_ in range(2):
                nc.sync.dma_start(
                    hn_out.ap().rearrange("p (k b) -> p k b", k=2)
                    [:, :, ds(s_ * SW, SW)], h_bf[s_][:])

    nc.compile()
    return nc


def scan_host_inputs(seq, emb, emb_purpose, w_ih, w_hh, b_ih, b_hh):
    seq = np.asarray(seq)
    xg = emb[seq]                      # [S, B, D] gather (input staging)
    in_maps = []
    for c in range(NCORES):
        p, h = CORE_PH[c]
        sl = slice(h * BH, (h + 1) * BH)
        xh = xg[:, sl, :]              # [S, BH, D]
        xT = np.ascontiguousarray(
            xh.transpose(2, 0, 1).reshape(2, 128, SB).transpose(1, 0, 2))
        wiT = np.ascontiguousarray(
            w_ih[p].T.reshape(2, 128, 768).transpose(1, 0, 2))
        whT = np.ascontiguousarray(
            w_hh[p].T.reshape(2, 128, 768).transpose(1, 0, 2))
        perm = [p, (p + 1) % 3, (p + 2) % 3]
        pT = np.ascontiguousarray(
            emb_purpose[perm].T.reshape(2, 128, 3).transpose(1, 0, 2))
        m = (seq[:, sl] != 0).astype(np.float32).reshape(SB)
        mask = np.ascontiguousarray(m.reshape(SB // 128, 128).T)
        bsum = (b_ih[p] + b_hh[p])[:512]
        in_maps.append({
            "wiT": wiT.astype(_BF), "whT": whT.astype(_BF),
            "xT": xT.astype(_BF),
            "pT": pT.astype(_BF), "mask": mask,
            "bri": bsum[None, :].astype(_BF),
            "bin": b_ih[p][None, 512:].astype(_BF),
            "bhn": np.ascontiguousarray(
                b_hh[p][512:].reshape(2, 128).T).astype(np.float32),
        })
    return in_maps


# --------------------------------------------------------------------------
# Launch 2: scoring
# --------------------------------------------------------------------------

def build_score_nc():
    nc = bacc.Bacc("TRN2", target_bir_lowering=False, debug=False,
                   num_devices=NCORES)

    hT6_d = nc.dram_tensor("hT6", [128, 6, 128], BF16, kind="ExternalInput")
    eT_d = nc.dram_tensor("eT", [128, 2, T_PAD], BF16, kind="ExternalInput")
    pT_d = nc.dram_tensor("pTs", [128, 2, 3], BF16, kind="ExternalInput")
    sel_d = nc.dram_tensor("sel", [128, 4, 128], BF16, kind="ExternalInput")
    scores_d = nc.dram_tensor("scores", [128, T_PAD], F32, kind="ExternalOutput")

    with tile.TileContext(nc) as tc:
        with (
            tc.tile_pool(name="consts", bufs=1) as consts,
            tc.tile_pool(name="spsum", bufs=1, space="PSUM") as spsum,
            tc.tile_pool(name="epsum", bufs=1, space="PSUM") as epsum,
            tc.tile_pool(name="work", bufs=4) as work,
            tc.tile_pool(name="outp", bufs=4) as outp,
        ):
            hT6 = consts.tile_from(hT6_d.ap())
            eT = consts.tile_from(eT_d.ap())
            pT = consts.tile_from(pT_d.ap())
            sel = consts.tile_from(sel_d.ap())

            # exp(logits), rows 0:3 only (K=3 matmuls read just those rows)
            sE = consts.tile([128, T_PAD], BF16)

            for ci, (c0, cs) in enumerate(zip(CHUNK_OFFS[:-1], CHUNK_SIZES)):
                ps_s = spsum.tile([128, N_CHUNK], F32, tag="logit",
                                  name="ps_s")
                for k in range(2):
                    nc.tensor.matmul(
                        ps_s[0:3, :cs], pT[:, k, :], eT[:, k, ds(c0, cs)],
                        start=(k == 0), stop=(k == 1))
                nc.scalar.activation(sE[0:3, ds(c0, cs)], ps_s[0:3, :cs], AF.Exp)

            for ci, (c0, cs) in enumerate(zip(CHUNK_OFFS[:-1], CHUNK_SIZES)):
                ps_P = [epsum.tile([128, N_CHUNK], F32, tag=f"P{p}",
                                   name=f"psP{p}") for p in range(3)]
                for p in range(3):
                    for k in range(2):
                        nc.tensor.matmul(
                            ps_P[p][:, :cs], hT6[:, p * 2 + k, :],
                            eT[:, k, ds(c0, cs)], start=(k == 0), stop=(k == 1))
                ps_E = [epsum.tile([128, N_CHUNK], F32, tag=f"E{j}",
                                   name=f"psE{j}") for j in range(4)]
                for j in range(4):
                    nc.tensor.matmul(
                        ps_E[j][:, :cs], sel[0:3, j, :], sE[0:3, ds(c0, cs)],
                        start=True, stop=True)
                den_f = work.tile([128, N_CHUNK], F32, tag="denf", name="den_f")
                nc.scalar.copy(den_f[:, :cs], ps_E[3][:, :cs])
                rden = work.tile([128, N_CHUNK], F32, tag="rden", name="rden")
                nc.vector.reciprocal_approx_fast(rden[:, :cs], den_f[:, :cs])
                e_sb = [work.tile([128, N_CHUNK], BF16, tag=f"esb{p}",
                                  name=f"esb{p}") for p in range(3)]
                nc.scalar.copy(e_sb[0][:, :cs], ps_E[0][:, :cs])
                nc.scalar.copy(e_sb[1][:, :cs], ps_E[1][:, :cs])
                nc.vector.tensor_copy(e_sb[2][:, :cs], ps_E[2][:, :cs])
                t_p = [work.tile([128, N_CHUNK], BF16, tag=f"tp{p}",
                                 name=f"tp{p}") for p in range(3)]
                for p in range(3):
                    nc.vector.tensor_tensor(
                        t_p[p][:, :cs], ps_P[p][:, :cs], e_sb[p][:, :cs],
                        OP.mult)
                s01 = work.tile([128, N_CHUNK], BF16, tag="s01", name="s01")
                nc.gpsimd.tensor_tensor(
                    s01[:, :cs], t_p[0][:, :cs], t_p[1][:, :cs], OP.add)
                s012 = work.tile([128, N_CHUNK], BF16, tag="s012", name="s012")
                nc.gpsimd.tensor_tensor(
                    s012[:, :cs], s01[:, :cs], t_p[2][:, :cs], OP.add)
                out_c = outp.tile([128, N_CHUNK], F32, tag="out", name="out_c")
                nc.gpsimd.tensor_tensor(
                    out_c[:, :cs], s012[:, :cs], rden[:, :cs], OP.mult)
                nc.sync.dma_start(scores_d.ap()[:, ds(c0, cs)], out_c[:, :cs])

    nc.compile()
    return nc


def score_host_inputs(hn_bf, emb, emb_purpose):
    embT = emb.T.astype(_BF)  # [256, 50001]
    pT = np.ascontiguousarray(
        emb_purpose.T.reshape(2, 128, 3).transpose(1, 0, 2)).astype(_BF)
    sel = np.zeros((128, 4, 128), np.float32)
    for p in range(3):
        sel[p, p, :] = 1.0
        sel[p, 3, :] = 1.0
    sel = sel.astype(_BF)

    base = N_ITEMS // NCORES
    rem = N_ITEMS - base * NCORES
    bounds = []
    s0 = 0
    for c in range(NCORES):
        n = base + (1 if c < rem else 0)
        bounds.append((s0, s0 + n))
        s0 += n

    in_maps = []
    for c in range(NCORES):
        lo, hi = bounds[c]
        n = hi - lo
        eT = np.zeros((128, 2, T_PAD), _BF)
        chunk = embT[:, lo:hi]
        eT[:, :, :n] = chunk.reshape(2, 128, n).transpose(1, 0, 2)
        in_maps.append({"hT6": hn_bf, "eT": eT, "pTs": pT, "sel": sel})
    return in_maps, bounds


# --------------------------------------------------------------------------
# Entry point
# --------------------------------------------------------------------------

_SCAN_NC = None
_SCORE_NC = None


def _get_ncs():
    global _SCAN_NC, _SCORE_NC
    if _SCAN_NC is None:
        _SCAN_NC = build_scan_nc()
    if _SCORE_NC is None:
        _SCORE_NC = build_score_nc()
    return _SCAN_NC, _SCORE_NC


def kernel(seq, emb, emb_purpose, w_ih, w_hh, b_ih, b_hh):
    seq = np.asarray(seq)
    emb = np.asarray(emb, np.float32)
    emb_purpose = np.asarray(emb_purpose, np.float32)
    w_ih = np.asarray(w_ih, np.float32)
    w_hh = np.asarray(w_hh, np.float32)
    b_ih = np.asarray(b_ih, np.float32)
    b_hh = np.asarray(b_hh, np.float32)

    scan_nc, score_nc = _get_ncs()

    scan_ins = scan_host_inputs(seq, emb, emb_purpose, w_ih, w_hh, b_ih, b_hh)
    res1 = run_bass_kernel_spmd(scan_nc, scan_ins, core_ids=list(range(NCORES)))

    hT6 = np.zeros((128, 6, 128), _BF)
    for c in range(6):
        p, h = CORE_PH[c]
        sl = res1.results[c]["hn_out"].reshape(128, 2, BH)
        for k in range(2):
            hT6[:, p * 2 + k, h * BH:(h + 1) * BH] = sl[:, k, :]

    score_ins, bounds = score_host_inputs(hT6, emb, emb_purpose)
    res2 = run_bass_kernel_spmd(score_nc, score_ins, core_ids=list(range(NCORES)))

    scores = np.empty((B, N_ITEMS), np.float32)
    for c in range(NCORES):
        lo, hi = bounds[c]
        scores[:, lo:hi] = res2.results[c]["scores"][:, : hi - lo]
    return scores


# revision 25
# speedup vs baseline: 1.2888x; 1.0445x over previous
"""Trainium2 Bass kernel for MCPRN (purpose-routed GRU-variant session recommender).

Pipeline (two SPMD launches on 8 NeuronCores):
  Launch 1 (scan): cores run (purpose p, batch-half h) PSRU scans, B_local=64.
     6 real slots + 2 duplicates. bf16 matmuls, fp32 elementwise state.
     Biases enter PSUM via K=1 ones-matmuls; the x-side (input) matmuls are
     batched 4 steps at a time (N=256) and the recurrent matmuls accumulate
     into the same PSUM group. Concentration weights (softmax over purposes
     of x . emb_purpose / tau, masked, eps-clamped) are computed on device in
     fp32 and broadcast across partitions via a DRAM roundtrip.
  Host gathers final hidden states hn[3, 128, 256] (bf16).
  Launch 2 (score): cores each score an item chunk (~6250 of 50001 items):
     scores[b, t] = sum_p tcw[t, p] * <hn[p, b, :], emb[t, :]>,
     tcw = softmax_p(emb @ emb_purpose.T). Softmax weights are broadcast
     across partitions with ones-matmuls; combine on DVE/GPSIMD.
"""

import numpy as np
import ml_dtypes

import concourse.bacc as bacc
import concourse.mybir as mybir
import concourse.tile as tile
from concourse.bass import ts, ds
from concourse.bass_utils import run_bass_kernel_spmd

F32 = mybir.dt.float32
BF16 = mybir.dt.bfloat16
AF = mybir.ActivationFunctionType
OP = mybir.AluOpType

N_ITEMS = 50001
DIM = 256
TAU = 0.1
S = 50
B = 128
EPS = 0.01
BH = 64         # batch half per scan core
SB = S * BH     # 3200 (step, batch) elements per scan core
NCORES = 8
GS = 8          # steps per x-side matmul group (8*64 = 512 f32 = 1 PSUM bank)
GROUPS = [(g, min(GS, S - g)) for g in range(0, S, GS)]

# scoring chunking
T_PAD = 6272            # 49 * 128, per-core padded item count
N_CHUNK = 512
CHUNK_SIZES = [512] * 12 + [128]
CHUNK_OFFS = np.cumsum([0] + CHUNK_SIZES).tolist()

CORE_PH = [(0, 0), (0, 1), (1, 0), (1, 1), (2, 0), (2, 1), (0, 0), (0, 1)]

_BF = ml_dtypes.bfloat16


# --------------------------------------------------------------------------
# Launch 1: scan
# --------------------------------------------------------------------------

def build_scan_nc():
    nc = bacc.Bacc("TRN2", target_bir_lowering=False, debug=False,
                   num_devices=NCORES)

    wiT_d = nc.dram_tensor("wiT", [128, 2, 768], BF16, kind="ExternalInput")
    whT_d = nc.dram_tensor("whT", [128, 2, 768], BF16, kind="ExternalInput")
    xT_d = nc.dram_tensor("xT", [128, 2, SB], BF16, kind="ExternalInput")
    pT_d = nc.dram_tensor("pT", [128, 2, 3], BF16, kind="ExternalInput")
    mask_d = nc.dram_tensor("mask", [128, SB // 128], F32, kind="ExternalInput")
    # bias rows (K=1 matmul stationary operands), bf16
    bri_d = nc.dram_tensor("bri", [1, 512], BF16, kind="ExternalInput")
    bin_d = nc.dram_tensor("bin", [1, 256], BF16, kind="ExternalInput")
    bhn_d = nc.dram_tensor("bhn", [128, 2], F32, kind="ExternalInput")
    hn_out = nc.dram_tensor("hn_out", [128, 2 * BH], BF16, kind="ExternalOutput")
    cf_lin = nc.dram_tensor("cf_lin", [SB // 128, 128], BF16)

    NT = SB // 128  # 25 (s,b)-tiles for concen

    with tile.TileContext(nc) as tc:
        with (
            tc.tile_pool(name="consts", bufs=1) as consts,
            tc.tile_pool(name="cw", bufs=1) as cw,
            tc.tile_pool(name="gx", bufs=1, space="PSUM") as gx,
            tc.tile_pool(name="ghn", bufs=1, space="PSUM") as ghnp,
            tc.tile_pool(name="ew", bufs=4) as ew,
            tc.tile_pool(name="hpool", bufs=3) as hpool,
        ):
            pT = consts.tile_from(pT_d.ap())
            xT = consts.tile_from(xT_d.ap())
            wiT = consts.tile_from(wiT_d.ap())
            whT = consts.tile_from(whT_d.ap())
            mask = consts.tile_from(mask_d.ap())
            bri = consts.tile_from(bri_d.ap())
            bin_ = consts.tile_from(bin_d.ap())
            bhn = consts.tile_from(bhn_d.ap())
            ones = consts.tile([1, GS * BH], BF16)
            nc.vector.memset(ones[:], 1.0)

            # ---------------- concen -> cf_rep ----------------
            ps_s = ghnp.tile([128, NT, 3], F32, tag="ghn0", name="ps_s")
            for tt in range(NT):
                for k in range(2):
                    nc.tensor.matmul(
                        ps_s[:, tt, :], xT[:, k, ts(tt, 128)], pT[:, k, :],
                        start=(k == 0), stop=(k == 1))
            e_s = cw.tile([128, NT, 3], F32)
            nc.scalar.activation(e_s[:], ps_s[:], AF.Exp, scale=1.0 / TAU)
            den = cw.tile([128, NT], F32)
            nc.vector.tensor_reduce(den[:], e_s[:], mybir.AxisListType.X, OP.add)
            rden = cw.tile([128, NT], F32)
            nc.vector.reciprocal_approx_fast(rden[:], den[:])
            cnorm = cw.tile([128, NT, 3], F32)
            nc.vector.tensor_tensor(
                cnorm[:], e_s[:],
                rden[:, :, None].to_broadcast((128, NT, 3)), OP.mult)
            cm = cw.tile([128, NT, 3], F32)
            nc.vector.tensor_tensor(
                cm[:], cnorm[:],
                mask[:, :, None].to_broadcast((128, NT, 3)), OP.mult)
            ge = cw.tile([128, NT, 3], F32)
            nc.vector.tensor_scalar(ge[:], cm[:], EPS, None, OP.is_ge)
            cf3 = cw.tile([128, NT, 3], BF16)
            nc.vector.tensor_tensor(cf3[:], cm[:], ge[:], OP.mult)
            cf_p = cf3[:, :, 0]  # host permutes purposes: col 0 = this core's

            nc.sync.dma_start(cf_lin.ap().rearrange("t p -> p t"), cf_p)
            cf_rep = cw.tile([128, SB], BF16)
            nc.sync.dma_start(
                cf_rep[:],
                cf_lin.ap().rearrange("t p -> (t p)")[None, :]
                .to_broadcast((128, SB)))

            # ---------------- the scan ----------------
            # two independent B=32 sub-scans (batch quarters) interleave so
            # each dependency chain hides in the other's bubbles; recurrent
            # matmuls share LDWEIGHTS between the subs.
            SW = BH // 2  # 32
            h = []
            for s_ in range(2):
                hs = hpool.tile([128, 2, SW], BF16, tag=f"h{s_}",
                                name=f"h_init{s_}")
                nc.vector.memset(hs[:], 0.0)
                h.append(hs)

            for g0, gn in GROUPS:
                gw = gn * BH
                # x-side matmuls for the whole group, bias seeded via K=1 mm
                g_ri = gx.tile([128, 4, GS, BH], F32, tag="gri", name="g_ri")
                g_in = gx.tile([128, 2, GS, BH], F32, tag="gin", name="g_in")
                for j in range(4):
                    nc.tensor.matmul(
                        g_ri[:, j, :gn, :], bri[0:1, ts(j, 128)], ones[0:1, :gw],
                        start=True, stop=False)
                    for k in range(2):
                        nc.tensor.matmul(
                            g_ri[:, j, :gn, :], wiT[:, k, ts(j, 128)],
                            xT[:, k, ds(g0 * BH, gw)], start=False,
                            stop=(k == 1))
                for j in range(2):
                    nc.tensor.matmul(
                        g_in[:, j, :gn, :], bin_[0:1, ts(j, 128)], ones[0:1, :gw],
                        start=True, stop=False)
                    for k in range(2):
                        nc.tensor.matmul(
                            g_in[:, j, :gn, :], wiT[:, k, ds(512 + j * 128, 128)],
                            xT[:, k, ds(g0 * BH, gw)], start=False,
                            stop=(k == 1))  # noqa: E501

                for tl in range(gn):
                    t = g0 + tl
                    # interleaved sub-scans: A's block fully precedes B's so
                    # PSUM-bank WAR deps stagger the chains by half a step
                    for s_ in range(2):
                        bsl = ds(s_ * SW, SW)
                        ps_ghn = ghnp.tile([128, 2, SW], F32, tag=f"ghn{s_}",
                                           name=f"ps_ghn{s_}")
                        for j in range(4):
                            for k in range(2):
                                nc.tensor.matmul(
                                    g_ri[:, j, tl, bsl],
                                    whT[:, k, ts(j, 128)],
                                    h[s_][:, k, :], start=False, stop=False,
                                    skip_group_check=True)
                        for j in range(2):
                            for k in range(2):
                                nc.tensor.matmul(
                                    ps_ghn[:, j, :],
                                    whT[:, k, ds(512 + j * 128, 128)],
                                    h[s_][:, k, :], start=(k == 0),
                                    stop=(k == 1))

                        ri_bf = ew.tile([128, 4, SW], BF16, tag=f"ri{s_}",
                                        name=f"ri_bf{s_}")
                        nc.scalar.activation(ri_bf[:], g_ri[:, :, tl, bsl],
                                             AF.Sigmoid)
                        u1 = ew.tile([128, 2, SW], F32, tag=f"u1{s_}",
                                     name=f"u1_{s_}")
                        for j in range(2):
                            nc.vector.scalar_tensor_tensor(
                                u1[:, j, :], ps_ghn[:, j, :], bhn[:, j:j + 1],
                                ri_bf[:, j, :], OP.add, OP.mult)
                        u2 = ew.tile([128, 2, SW], F32, tag=f"u2{s_}",
                                     name=f"u2_{s_}")
                        nc.vector.tensor_tensor(u2[:], u1[:],
                                                g_in[:, :, tl, bsl], OP.add)
                        n_t = ew.tile([128, 2, SW], F32, tag=f"n{s_}",
                                      name=f"n_t{s_}")
                        nc.scalar.activation(n_t[:], u2[:], AF.Tanh)
                        a_t = ew.tile([128, 2, SW], F32, tag=f"a{s_}",
                                      name=f"a_t{s_}")
                        nc.gpsimd.tensor_tensor(
                            a_t[:], ri_bf[:, 2:4, :],
                            cf_rep[:, None, ds(t * BH + s_ * SW, SW)]
                            .to_broadcast((128, 2, SW)), OP.mult)
                        q_t = ew.tile([128, 2, SW], F32, tag=f"q{s_}",
                                      name=f"q_t{s_}")
                        nc.vector.tensor_scalar(q_t[:], a_t[:], -1.0, 1.0,
                                                OP.mult, OP.add)
                        hq = ew.tile([128, 2, SW], F32, tag=f"hqt{s_}",
                                     name=f"hq{s_}")
                        nc.gpsimd.tensor_tensor(hq[:], h[s_][:], q_t[:],
                                                OP.mult)
                        an = ew.tile([128, 2, SW], F32, tag=f"ant{s_}",
                                     name=f"an{s_}")
                        nc.vector.tensor_tensor(an[:], a_t[:], n_t[:], OP.mult)
                        h_new = hpool.tile([128, 2, SW], BF16, tag=f"h{s_}",
                                           name=f"h_new{s_}")
                        nc.vector.tensor_tensor(h_new[:], hq[:], an[:],
                                                OP.add)
                        h[s_] = h_new

            for s_ in range(2):
                nc.sync.dma_start(
                    hn_out.ap().rearrange("p (k b) -> p k b", k=2)
                    [:, :, ds(s_ * SW, SW)], h[s_][:])

    nc.compile()
    return nc


def scan_host_inputs(seq, emb, emb_purpose, w_ih, w_hh, b_ih, b_hh):
    seq = np.asarray(seq)
    xg = emb[seq]                      # [S, B, D] gather (input staging)
    in_maps = []
    for c in range(NCORES):
        p, h = CORE_PH[c]
        sl = slice(h * BH, (h + 1) * BH)
        xh = xg[:, sl, :]              # [S, BH, D]
        xT = np.ascontiguousarray(
            xh.transpose(2, 0, 1).reshape(2, 128, SB).transpose(1, 0, 2))
        wiT = np.ascontiguousarray(
            w_ih[p].T.reshape(2, 128, 768).transpose(1, 0, 2))
        whT = np.ascontiguousarray(
            w_hh[p].T.reshape(2, 128, 768).transpose(1, 0, 2))
        perm = [p, (p + 1) % 3, (p + 2) % 3]
        pT = np.ascontiguousarray(
            emb_purpose[perm].T.reshape(2, 128, 3).transpose(1, 0, 2))
        m = (seq[:, sl] != 0).astype(np.float32).reshape(SB)
        mask = np.ascontiguousarray(m.reshape(SB // 128, 128).T)
        bsum = (b_ih[p] + b_hh[p])[:512]
        in_maps.append({
            "wiT": wiT.astype(_BF), "whT": whT.astype(_BF),
            "xT": xT.astype(_BF),
            "pT": pT.astype(_BF), "mask": mask,
            "bri": bsum[None, :].astype(_BF),
            "bin": b_ih[p][None, 512:].astype(_BF),
            "bhn": np.ascontiguousarray(
                b_hh[p][512:].reshape(2, 128).T).astype(np.float32),
        })
    return in_maps


# --------------------------------------------------------------------------
# Launch 2: scoring
# --------------------------------------------------------------------------

def build_score_nc():
    nc = bacc.Bacc("TRN2", target_bir_lowering=False, debug=False,
                   num_devices=NCORES)

    hT6_d = nc.dram_tensor("hT6", [128, 6, 128], BF16, kind="ExternalInput")
    eT_d = nc.dram_tensor("eT", [128, 2, T_PAD], BF16, kind="ExternalInput")
    pT_d = nc.dram_tensor("pTs", [128, 2, 3], BF16, kind="ExternalInput")
    sel_d = nc.dram_tensor("sel", [128, 4, 128], BF16, kind="ExternalInput")
    scores_d = nc.dram_tensor("scores", [128, T_PAD], F32, kind="ExternalOutput")

    with tile.TileContext(nc) as tc:
        with (
            tc.tile_pool(name="consts", bufs=1) as consts,
            tc.tile_pool(name="spsum", bufs=1, space="PSUM") as spsum,
            tc.tile_pool(name="epsum", bufs=1, space="PSUM") as epsum,
            tc.tile_pool(name="work", bufs=4) as work,
            tc.tile_pool(name="outp", bufs=4) as outp,
        ):
            hT6 = consts.tile_from(hT6_d.ap())
            eT = consts.tile_from(eT_d.ap())
            pT = consts.tile_from(pT_d.ap())
            sel = consts.tile_from(sel_d.ap())

            # exp(logits), rows 0:3 only (K=3 matmuls read just those rows)
            sE = consts.tile([128, T_PAD], BF16)

            for ci, (c0, cs) in enumerate(zip(CHUNK_OFFS[:-1], CHUNK_SIZES)):
                ps_s = spsum.tile([128, N_CHUNK], F32, tag="logit",
                                  name="ps_s")
                for k in range(2):
                    nc.tensor.matmul(
                        ps_s[0:3, :cs], pT[:, k, :], eT[:, k, ds(c0, cs)],
                        start=(k == 0), stop=(k == 1))
                nc.scalar.activation(sE[0:3, ds(c0, cs)], ps_s[0:3, :cs], AF.Exp)

            for ci, (c0, cs) in enumerate(zip(CHUNK_OFFS[:-1], CHUNK_SIZES)):
                ps_P = [epsum.tile([128, N_CHUNK], F32, tag=f"P{p}",
                                   name=f"psP{p}") for p in range(3)]
                for p in range(3):
                    for k in range(2):
                        nc.tensor.matmul(
                            ps_P[p][:, :cs], hT6[:, p * 2 + k, :],
                            eT[:, k, ds(c0, cs)], start=(k == 0), stop=(k == 1))
                ps_E = [epsum.tile([128, N_CHUNK], F32, tag=f"E{j}",
                                   name=f"psE{j}") for j in range(4)]
                for j in range(4):
                    nc.tensor.matmul(
                        ps_E[j][:, :cs], sel[0:3, j, :], sE[0:3, ds(c0, cs)],
                        start=True, stop=True)
                rden = work.tile([128, N_CHUNK], F32, tag="rden", name="rden")
                nc.vector.reciprocal_approx_fast(rden[:, :cs], ps_E[3][:, :cs])
                e_sb = [work.tile([128, N_CHUNK], BF16, tag=f"esb{p}",
                                  name=f"esb{p}") for p in range(3)]
                nc.scalar.copy(e_sb[0][:, :cs], ps_E[0][:, :cs])
                nc.scalar.copy(e_sb[1][:, :cs], ps_E[1][:, :cs])
                nc.vector.tensor_copy(e_sb[2][:, :cs], ps_E[2][:, :cs])
                t_p = [work.tile([128, N_CHUNK], BF16, tag=f"tp{p}",
                                 name=f"tp{p}") for p in range(3)]
                for p in range(3):
                    nc.vector.tensor_tensor(
                        t_p[p][:, :cs], ps_P[p][:, :cs], e_sb[p][:, :cs],
                        OP.mult)
                s01 = work.tile([128, N_CHUNK], BF16, tag="s01", name="s01")
                nc.gpsimd.tensor_tensor(
                    s01[:, :cs], t_p[0][:, :cs], t_p[1][:, :cs], OP.add)
                s012 = work.tile([128, N_CHUNK], BF16, tag="s012", name="s012")
                nc.gpsimd.tensor_tensor(
                    s012[:, :cs], s01[:, :cs], t_p[2][:, :cs], OP.add)
                out_c = outp.tile([128, N_CHUNK], F32, tag="out", name="out_c")
                nc.gpsimd.tensor_tensor(
                    out_c[:, :cs], s012[:, :cs], rden[:, :cs], OP.mult)
                nc.sync.dma_start(scores_d.ap()[:, ds(c0, cs)], out_c[:, :cs])

    nc.compile()
    return nc


def score_host_inputs(hn_bf, emb, emb_purpose):
    embT = emb.T.astype(_BF)  # [256, 50001]
    pT = np.ascontiguousarray(
        emb_purpose.T.reshape(2, 128, 3).transpose(1, 0, 2)).astype(_BF)
    sel = np.zeros((128, 4, 128), np.float32)
    for p in range(3):
        sel[p, p, :] = 1.0
        sel[p, 3, :] = 1.0
    sel = sel.astype(_BF)

    base = N_ITEMS // NCORES
    rem = N_ITEMS - base * NCORES
    bounds = []
    s0 = 0
    for c in range(NCORES):
        n = base + (1 if c < rem else 0)
        bounds.append((s0, s0 + n))
        s0 += n

    in_maps = []
    for c in range(NCORES):
        lo, hi = bounds[c]
        n = hi - lo
        eT = np.zeros((128, 2, T_PAD), _BF)
        chunk = embT[:, lo:hi]
        eT[:, :, :n] = chunk.reshape(2, 128, n).transpose(1, 0, 2)
        in_maps.append({"hT6": hn_bf, "eT": eT, "pTs": pT, "sel": sel})
    return in_maps, bounds


# --------------------------------------------------------------------------
# Entry point
# --------------------------------------------------------------------------

_SCAN_NC = None
_SCORE_NC = None


def _get_ncs():
    global _SCAN_NC, _SCORE_NC
    if _SCAN_NC is None:
        _SCAN_NC = build_scan_nc()
    if _SCORE_NC is None:
        _SCORE_NC = build_score_nc()
    return _SCAN_NC, _SCORE_NC


def kernel(seq, emb, emb_purpose, w_ih, w_hh, b_ih, b_hh):
    seq = np.asarray(seq)
    emb = np.asarray(emb, np.float32)
    emb_purpose = np.asarray(emb_purpose, np.float32)
    w_ih = np.asarray(w_ih, np.float32)
    w_hh = np.asarray(w_hh, np.float32)
    b_ih = np.asarray(b_ih, np.float32)
    b_hh = np.asarray(b_hh, np.float32)

    scan_nc, score_nc = _get_ncs()

    scan_ins = scan_host_inputs(seq, emb, emb_purpose, w_ih, w_hh, b_ih, b_hh)
    res1 = run_bass_kernel_spmd(scan_nc, scan_ins, core_ids=list(range(NCORES)))

    hT6 = np.zeros((128, 6, 128), _BF)
    for c in range(6):
        p, h = CORE_PH[c]
        sl = res1.results[c]["hn_out"].reshape(128, 2, BH)
        for k in range(2):
            hT6[:, p * 2 + k, h * BH:(h + 1) * BH] = sl[:, k, :]

    score_ins, bounds = score_host_inputs(hT6, emb, emb_purpose)
    res2 = run_bass_kernel_spmd(score_nc, score_ins, core_ids=list(range(NCORES)))

    scores = np.empty((B, N_ITEMS), np.float32)
    for c in range(NCORES):
        lo, hi = bounds[c]
        scores[:, lo:hi] = res2.results[c]["scores"][:, : hi - lo]
    return scores
